# revision 1
# baseline (speedup 1.0000x reference)
"""AlphaRotatedIoULoss on 8 TRN2 NeuronCores (raw Bass SPMD kernel, v2).

Sort-free replication of the reference's rotated-IoU loss:
  - intersection area via directed-segment shoelace: clip each box's 4 edges
    against the other box (branch-free Liang-Barsky in that box's local
    frame), then sum span*cross(k,d) over the 8 directed boundary segments
    (all expressed in box2's frame).
  - the reference's shoelace drops the closing edge (last->first angle-sorted
    vertex) because invalid candidate slots are zeroed; the missing term is
    the cross of the unique boundary segment crossing the global -x ray from
    the vertex centroid.  Replicated branch-free via a global-y sign test.
Data-parallel over boxes: each core takes 1/8th, emits per-partition
per-chunk partial sums of weight*iou^3; host:  loss = (sum(w) - total) / n.

v2: F=248 (2 chunks) with manually aliased SBUF scratch, scalings folded
into scalar_tensor_tensor ops, product negations written straight into the
segment-direction slices, analytic cross terms for box2's own (axis-aligned)
edges.
"""
import numpy as np

P = 128          # partitions
T = 496          # boxes per partition per core
F = 248          # chunk width (free dim)
NCHUNK = T // F  # 2
NCORE = P * T    # 63488 boxes per core
NSHARD = 62500   # real boxes per core
NGLOB = 500000
NCORES = 8
ALPHA_EPS = 1e-6
TINY = 1e-12

_cache = {}


def _build_graph():
    import concourse.bass as bass
    import concourse.mybir as mybir
    from contextlib import ExitStack

    dt = mybir.dt.float32
    AF = mybir.ActivationFunctionType
    OP = mybir.AluOpType
    AX = mybir.AxisListType
    HALF_PI = float(np.pi / 2)

    nc = bass.Bass(detect_race_conditions=False)

    def reg_const(value):
        t = nc.alloc_sbuf_tensor(f"const-f32-{value}", [128, 1], dt)
        nc.gpsimd.memset(t.ap(), value)
        nc.const_aps.aps[(dt, float(value))] = t.ap()

    reg_const(HALF_PI)
    nc.all_engine_barrier()

    pred = nc.declare_dram_parameter("pred", [NCORE, 5], dt, isOutput=False)
    targ = nc.declare_dram_parameter("target", [NCORE, 5], dt, isOutput=False)
    wgt = nc.declare_dram_parameter("weight", [NCORE], dt, isOutput=False)
    out = nc.declare_dram_parameter("out", [P, NCHUNK], dt, isOutput=True)

    predv = pred.rearrange("(p t) f -> p t f", p=P)
    targv = targ.rearrange("(p t) f -> p t f", p=P)
    wv = wgt.rearrange("(p t) -> p t", p=P)

    V = nc.vector
    A = nc.scalar

    with ExitStack() as ctx:
        _n = [0]

        def alloc(shape):
            _n[0] += 1
            return ctx.enter_context(nc.sbuf_tensor(f"tile{_n[0]}", shape, dt))

        # double-buffered inputs (2 chunks -> one buffer each, no recycling)
        pt2 = [alloc([P, F, 5]) for _ in range(2)]
        tg2 = [alloc([P, F, 5]) for _ in range(2)]
        wt2 = [alloc([P, F]) for _ in range(2)]
        # ACT-owned trig outputs
        sdr_t, cd_t, s1_t, c1_t, s2_t, c2_t = (alloc([P, F]) for _ in range(6))
        ltmp = [alloc([P, F]) for _ in range(4)]
        # F-sized DVE scratch
        FN = ("delta tmpA tmpB px py W1 H1 W2 H2 ar1 ar2 ar2h sg asd sd "
              "vcf vsf gcf gsf mu mv nu nv E1f E2f F1f F2f P1f P2f Q1f Q2f "
              "awsf ahsf avsf agsf "
              "m_ sx sy i2m cx cy cgy S_ CR_ inter un iou io2 io3 junk").split()
        FT = {n: alloc([P, F]) for n in FN}
        # 4F tiles
        C4 = {n: alloc([P, 4 * F]) for n in
              "KXP KXM KYP KYM RXA RYA RXB RYB GX4 GY4".split()}
        # 8F tiles
        E8 = {n: alloc([P, 8 * F]) for n in
              "K8X K8Y D8X D8Y TLO THI SPAN EA EB EC ED".split()}
        acc4 = alloc([P, NCHUNK])

        with (
            nc.semaphore("dma_sem") as dma_sem,
            nc.semaphore("v_sem") as v_sem,
            nc.semaphore("a_sem") as a_sem,
            nc.semaphore("v2_sem") as v2_sem,
            nc.semaphore("a2_sem") as a2_sem,
            nc.semaphore("done_sem") as done_sem,
            nc.Block() as block,
        ):
            @block.sync
            def _(sync):
                for ch in range(NCHUNK):
                    sync.dma_start(
                        out=pt2[ch][:], in_=predv[:, ch * F:(ch + 1) * F, :]
                    ).then_inc(dma_sem, 16)
                    sync.dma_start(
                        out=tg2[ch][:], in_=targv[:, ch * F:(ch + 1) * F, :]
                    ).then_inc(dma_sem, 16)
                    sync.dma_start(
                        out=wt2[ch][:], in_=wv[:, ch * F:(ch + 1) * F]
                    ).then_inc(dma_sem, 16)
                sync.wait_ge(done_sem, 1)
                sync.dma_start(out=out[:], in_=acc4[:]).then_inc(dma_sem, 16)

            @block.scalar
            def _(scalar):
                for ch in range(NCHUNK):
                    a1 = pt2[ch][:, :, 4]
                    a2 = tg2[ch][:, :, 4]
                    scalar.wait_ge(v_sem, ch + 1)
                    A.activation(sdr_t[:], FT["delta"][:], AF.Sin)
                    A.activation(cd_t[:], FT["delta"][:], AF.Sin, bias=HALF_PI)
                    A.activation(s1_t[:], a1, AF.Sin)
                    A.activation(c1_t[:], a1, AF.Sin, bias=HALF_PI)
                    A.activation(s2_t[:], a2, AF.Sin)
                    A.activation(c2_t[:], FT["tmpA"][:], AF.Sin)
                    A.drain().then_inc(a_sem, 1)
                    scalar.wait_ge(v2_sem, ch + 1)
                    # clip reciprocals: r = exp(-ln(x)), x > 0
                    wcf_s = E8["D8X"][:, 2 * F:3 * F]
                    hcf_s = E8["D8Y"][:, 3 * F:4 * F]
                    for ins, outs in (
                        ((wcf_s, FT["ahsf"][:], FT["awsf"][:], hcf_s),
                         (C4["RXA"][:, 0:F], C4["RXA"][:, 3 * F:4 * F],
                          C4["RYA"][:, 0:F], C4["RYA"][:, F:2 * F])),
                        ((FT["vcf"][:], FT["agsf"][:], FT["avsf"][:],
                          FT["gcf"][:]),
                         (C4["RXB"][:, 0:F], C4["RXB"][:, F:2 * F],
                          C4["RYB"][:, 2 * F:3 * F], C4["RYB"][:, F:2 * F])),
                    ):
                        for j in range(4):
                            A.activation(ltmp[j][:], ins[j], AF.Ln)
                        for j in range(4):
                            A.activation(outs[j], ltmp[j][:], AF.Exp,
                                         scale=-1.0)
                    A.drain().then_inc(a2_sem, 1)

            @block.vector
            def _(vector):
                t = lambda n: FT[n][:]
                c4 = lambda n: C4[n][:]
                e8 = lambda n: E8[n][:]

                def sl(nm, i):
                    return E8[nm][:, i * F:(i + 1) * F]

                def sl4(nm, i):
                    return C4[nm][:, i * F:(i + 1) * F]

                def segreduce(dst, nm):
                    v = E8[nm][:].rearrange("p (s f) -> p f s", s=8)
                    V.tensor_reduce(dst, v, AX.X, OP.add)

                # constant zero slices of D8X/D8Y (box2's own AA edge dirs)
                V.memset(sl("D8X", 5), 0.0)
                V.memset(sl("D8X", 7), 0.0)
                V.memset(sl("D8Y", 4), 0.0)
                V.memset(sl("D8Y", 6), 0.0)

                for ch in range(NCHUNK):
                    pt, tg, wt = pt2[ch], tg2[ch], wt2[ch]
                    x1, y1, w1, h1, a1 = (pt[:, :, i] for i in range(5))
                    x2, y2, w2, h2, a2 = (tg[:, :, i] for i in range(5))

                    vector.wait_ge(dma_sem, 48 * (ch + 1))
                    # angles for ACT: delta, and wrapped a2+pi/2 in tmpA
                    V.tensor_tensor(t("delta"), a1, a2, OP.subtract)
                    V.tensor_scalar(t("tmpA"), a2, HALF_PI, None, OP.add)
                    V.tensor_scalar(t("tmpB"), t("tmpA"), float(np.pi), None,
                                    OP.is_gt)
                    V.scalar_tensor_tensor(t("tmpA"), t("tmpB"),
                                           float(-2 * np.pi), t("tmpA"),
                                           OP.mult, OP.add)
                    V.drain().then_inc(v_sem, 1)

                    # trig-independent work
                    V.tensor_tensor(t("px"), x2, x1, OP.subtract)
                    V.tensor_tensor(t("py"), y2, y1, OP.subtract)
                    V.tensor_scalar(t("W1"), w1, 0.5, None, OP.mult)
                    V.tensor_scalar(t("H1"), h1, 0.5, None, OP.mult)
                    V.tensor_scalar(t("W2"), w2, 0.5, None, OP.mult)
                    V.tensor_scalar(t("H2"), h2, 0.5, None, OP.mult)
                    V.tensor_tensor(t("ar1"), w1, h1, OP.mult)
                    V.tensor_tensor(t("ar2"), w2, h2, OP.mult)
                    V.tensor_scalar(t("ar2h"), t("ar2"), 0.5, None, OP.mult)
                    # box2 own corners -> K8X/K8Y slices 4..7
                    V.tensor_copy(sl("K8X", 4), t("W2"))
                    V.tensor_scalar(sl("K8X", 5), t("W2"), -1.0, None, OP.mult)
                    V.tensor_copy(sl("K8X", 6), sl("K8X", 5))
                    V.tensor_copy(sl("K8X", 7), t("W2"))
                    V.tensor_copy(sl("K8Y", 4), t("H2"))
                    V.tensor_copy(sl("K8Y", 5), t("H2"))
                    V.tensor_scalar(sl("K8Y", 6), t("H2"), -1.0, None, OP.mult)
                    V.tensor_copy(sl("K8Y", 7), sl("K8Y", 6))
                    # box2 own edge dirs -> D8X/D8Y slices 4..7 (x: -w2,0,w2,0)
                    V.tensor_scalar(sl("D8X", 4), w2, -1.0, None, OP.mult)
                    V.tensor_scalar(sl("D8X", 6), sl("D8X", 4), -1.0, None,
                                    OP.mult)
                    V.tensor_scalar(sl("D8Y", 5), h2, -1.0, None, OP.mult)
                    V.tensor_scalar(sl("D8Y", 7), sl("D8Y", 5), -1.0, None,
                                    OP.mult)

                    # ---- trig-dependent ------------------------------------
                    vector.wait_ge(a_sem, ch + 1)
                    cdA, s1A, c1A, s2A, c2A = (cd_t[:], s1_t[:], c1_t[:],
                                               s2_t[:], c2_t[:])
                    V.tensor_scalar(t("sg"), sdr_t[:], 0.0, None, OP.is_ge)
                    V.tensor_scalar(t("sg"), t("sg"), 2.0, -1.0, OP.mult,
                                    OP.add)
                    V.tensor_tensor(t("asd"), t("sg"), sdr_t[:], OP.mult)
                    V.tensor_scalar(t("asd"), t("asd"), TINY, None, OP.max)
                    V.tensor_tensor(t("sd"), t("sg"), t("asd"), OP.mult)
                    sdA = t("sd")

                    # box1 full products straight into D8 slices 0..3
                    wcf = sl("D8X", 2); V.tensor_tensor(wcf, w1, cdA, OP.mult)
                    V.tensor_scalar(sl("D8X", 0), wcf, -1.0, None, OP.mult)
                    wsf = sl("D8Y", 2); V.tensor_tensor(wsf, w1, sdA, OP.mult)
                    V.tensor_scalar(sl("D8Y", 0), wsf, -1.0, None, OP.mult)
                    hsf = sl("D8X", 1); V.tensor_tensor(hsf, h1, sdA, OP.mult)
                    V.tensor_scalar(sl("D8X", 3), hsf, -1.0, None, OP.mult)
                    hcf = sl("D8Y", 3); V.tensor_tensor(hcf, h1, cdA, OP.mult)
                    V.tensor_scalar(sl("D8Y", 1), hcf, -1.0, None, OP.mult)
                    # box2 full products (kept as F tiles; clip-B recips)
                    V.tensor_tensor(t("vcf"), w2, cdA, OP.mult)
                    V.tensor_tensor(t("vsf"), w2, sdA, OP.mult)
                    V.tensor_tensor(t("gcf"), h2, cdA, OP.mult)
                    V.tensor_tensor(t("gsf"), h2, sdA, OP.mult)
                    # |.| products for the ACT reciprocals of signed inputs
                    V.tensor_tensor(t("awsf"), w1, t("asd"), OP.mult)
                    V.tensor_tensor(t("ahsf"), h1, t("asd"), OP.mult)
                    V.tensor_tensor(t("avsf"), w2, t("asd"), OP.mult)
                    V.tensor_tensor(t("agsf"), h2, t("asd"), OP.mult)
                    V.drain().then_inc(v2_sem, 1)

                    # centers
                    V.tensor_tensor(t("tmpA"), t("px"), c2A, OP.mult)
                    V.tensor_tensor(t("tmpB"), t("py"), s2A, OP.mult)
                    V.scalar_tensor_tensor(t("mu"), t("tmpA"), -1.0, t("tmpB"),
                                           OP.mult, OP.subtract)
                    V.tensor_tensor(t("tmpA"), t("px"), s2A, OP.mult)
                    V.tensor_tensor(t("tmpB"), t("py"), c2A, OP.mult)
                    V.tensor_tensor(t("mv"), t("tmpA"), t("tmpB"), OP.subtract)
                    V.tensor_tensor(t("tmpA"), t("px"), c1A, OP.mult)
                    V.tensor_tensor(t("tmpB"), t("py"), s1A, OP.mult)
                    V.tensor_tensor(t("nu"), t("tmpA"), t("tmpB"), OP.add)
                    V.tensor_tensor(t("tmpA"), t("px"), s1A, OP.mult)
                    V.tensor_tensor(t("tmpB"), t("py"), c1A, OP.mult)
                    V.tensor_tensor(t("nv"), t("tmpB"), t("tmpA"), OP.subtract)

                    # box1 corners in box2 frame (half-scales folded into STT)
                    V.tensor_tensor(t("E1f"), wcf, hsf, OP.subtract)
                    V.tensor_tensor(t("E2f"), wcf, hsf, OP.add)
                    V.tensor_tensor(t("F1f"), wsf, hcf, OP.add)
                    V.tensor_tensor(t("F2f"), wsf, hcf, OP.subtract)
                    V.scalar_tensor_tensor(sl("K8X", 0), t("E1f"), 0.5, t("mu"), OP.mult, OP.add)
                    V.scalar_tensor_tensor(sl("K8X", 1), t("E2f"), -0.5, t("mu"), OP.mult, OP.add)
                    V.scalar_tensor_tensor(sl("K8X", 2), t("E1f"), -0.5, t("mu"), OP.mult, OP.add)
                    V.scalar_tensor_tensor(sl("K8X", 3), t("E2f"), 0.5, t("mu"), OP.mult, OP.add)
                    V.scalar_tensor_tensor(sl("K8Y", 0), t("F1f"), 0.5, t("mv"), OP.mult, OP.add)
                    V.scalar_tensor_tensor(sl("K8Y", 1), t("F2f"), -0.5, t("mv"), OP.mult, OP.add)
                    V.scalar_tensor_tensor(sl("K8Y", 2), t("F1f"), -0.5, t("mv"), OP.mult, OP.add)
                    V.scalar_tensor_tensor(sl("K8Y", 3), t("F2f"), 0.5, t("mv"), OP.mult, OP.add)
                    # box2 corners in box1 frame
                    V.tensor_tensor(t("P1f"), t("vcf"), t("gsf"), OP.add)
                    V.tensor_tensor(t("P2f"), t("vcf"), t("gsf"), OP.subtract)
                    V.tensor_tensor(t("Q1f"), t("gcf"), t("vsf"), OP.subtract)
                    V.tensor_tensor(t("Q2f"), t("gcf"), t("vsf"), OP.add)
                    V.scalar_tensor_tensor(sl4("GX4", 0), t("P1f"), 0.5, t("nu"), OP.mult, OP.add)
                    V.scalar_tensor_tensor(sl4("GX4", 1), t("P2f"), -0.5, t("nu"), OP.mult, OP.add)
                    V.scalar_tensor_tensor(sl4("GX4", 2), t("P1f"), -0.5, t("nu"), OP.mult, OP.add)
                    V.scalar_tensor_tensor(sl4("GX4", 3), t("P2f"), 0.5, t("nu"), OP.mult, OP.add)
                    V.scalar_tensor_tensor(sl4("GY4", 0), t("Q1f"), 0.5, t("nv"), OP.mult, OP.add)
                    V.scalar_tensor_tensor(sl4("GY4", 1), t("Q2f"), 0.5, t("nv"), OP.mult, OP.add)
                    V.scalar_tensor_tensor(sl4("GY4", 2), t("Q1f"), -0.5, t("nv"), OP.mult, OP.add)
                    V.scalar_tensor_tensor(sl4("GY4", 3), t("Q2f"), -0.5, t("nv"), OP.mult, OP.add)

                    def b3(ft):
                        # [P,F] -> broadcast [P,4,F]
                        return ft.rearrange("p (o f) -> p o f", o=1)\
                                 .to_broadcast((P, 4, F))

                    def v3(ap4):
                        return ap4.rearrange("p (s f) -> p s f", s=4)

                    vector.wait_ge(a2_sem, ch + 1)

                    def emit_clip(corner_x4, corner_y4, Wb, Hb,
                                  RXn, RYn, sgn_slots,
                                  lo_out, hi_out, span_out):
                        # shifted corners, batched with broadcast bounds
                        V.tensor_tensor(v3(c4("KXP")), v3(corner_x4), b3(Wb),
                                        OP.add)
                        V.tensor_tensor(v3(c4("KXM")), v3(corner_x4), b3(Wb),
                                        OP.subtract)
                        V.tensor_tensor(v3(c4("KYP")), v3(corner_y4), b3(Hb),
                                        OP.add)
                        V.tensor_tensor(v3(c4("KYM")), v3(corner_y4), b3(Hb),
                                        OP.subtract)
                        # R tiles: ACT prefilled positive slices; apply sign
                        # to the signed ones in place, then fill negated slots
                        for Rt, (pos0, neg0, pos1, neg1, signed) in (
                                (RXn, sgn_slots[0]), (RYn, sgn_slots[1])):
                            for s in signed:
                                V.tensor_tensor(sl4(Rt, s), t("sg"),
                                                sl4(Rt, s), OP.mult)
                            V.tensor_scalar(sl4(Rt, neg0), sl4(Rt, pos0),
                                            -1.0, None, OP.mult)
                            V.tensor_scalar(sl4(Rt, neg1), sl4(Rt, pos1),
                                            -1.0, None, OP.mult)
                        V.tensor_tensor(c4("KXP"), c4("KXP"), c4(RXn), OP.mult)
                        V.tensor_tensor(c4("KXM"), c4("KXM"), c4(RXn), OP.mult)
                        V.tensor_tensor(c4("KYP"), c4("KYP"), c4(RYn), OP.mult)
                        V.tensor_tensor(c4("KYM"), c4("KYM"), c4(RYn), OP.mult)
                        V.tensor_tensor(c4(RXn), c4("KXP"), c4("KXM"), OP.min)
                        V.tensor_tensor(c4("KXP"), c4("KXP"), c4("KXM"), OP.max)
                        V.tensor_tensor(c4(RYn), c4("KYP"), c4("KYM"), OP.min)
                        V.tensor_tensor(c4("KYP"), c4("KYP"), c4("KYM"), OP.max)
                        V.tensor_tensor(lo_out, c4(RXn), c4(RYn), OP.max)
                        V.tensor_scalar(lo_out, lo_out, 0.0, None, OP.max)
                        V.tensor_tensor(hi_out, c4("KXP"), c4("KYP"), OP.min)
                        V.tensor_scalar(hi_out, hi_out, 1.0, None, OP.min)
                        V.scalar_tensor_tensor(span_out, lo_out, -1.0, hi_out,
                                               OP.mult, OP.add)
                        V.tensor_scalar(span_out, span_out, 0.0, None, OP.max)

                    # part A: box1 edges vs AA box2
                    # RXA: pos0=0(wcf,+) neg->2; pos1=3(|hsf|,signed) neg->1
                    # RYA: pos0=0(|wsf|,signed) neg->2; pos1=1(hcf,+) neg->3
                    emit_clip(E8["K8X"][:, 0:4 * F], E8["K8Y"][:, 0:4 * F],
                              t("W2"), t("H2"), "RXA", "RYA",
                              ((0, 2, 3, 1, (3,)), (0, 2, 1, 3, (0,))),
                              E8["TLO"][:, 0:4 * F], E8["THI"][:, 0:4 * F],
                              E8["SPAN"][:, 0:4 * F])
                    # part B: box2 edges vs AA box1
                    # RXB: pos0=0(vcf,+) neg->2; pos1=1(|gsf|,signed) neg->3
                    # RYB: pos0=2(|vsf|,signed) neg->0; pos1=1(gcf,+) neg->3
                    emit_clip(c4("GX4"), c4("GY4"),
                              t("W1"), t("H1"), "RXB", "RYB",
                              ((0, 2, 1, 3, (1,)), (2, 0, 1, 3, (2,))),
                              E8["TLO"][:, 4 * F:8 * F],
                              E8["THI"][:, 4 * F:8 * F],
                              E8["SPAN"][:, 4 * F:8 * F])

                    # ---- area terms ----------------------------------------
                    # A half: numeric cross(k,d); B half: cross = 2*W2*H2
                    EAa = E8["EA"][:, 0:4 * F]
                    EBa = E8["EB"][:, 0:4 * F]
                    V.tensor_tensor(EAa, E8["K8X"][:, 0:4 * F],
                                    E8["D8Y"][:, 0:4 * F], OP.mult)
                    V.tensor_tensor(EBa, E8["K8Y"][:, 0:4 * F],
                                    E8["D8X"][:, 0:4 * F], OP.mult)
                    V.tensor_tensor(EAa, EAa, EBa, OP.subtract)
                    V.tensor_tensor(EBa, E8["SPAN"][:, 0:4 * F], EAa, OP.mult)
                    V.tensor_tensor(
                        E8["EB"][:, 4 * F:8 * F].rearrange("p (s f) -> p s f", s=4),
                        E8["SPAN"][:, 4 * F:8 * F].rearrange("p (s f) -> p s f", s=4),
                        b3(t("ar2h")), OP.mult)
                    # EB = SV8 (span * cross) for all 8 segments

                    # ---- vertex centroid -----------------------------------
                    V.tensor_scalar(e8("EC"), e8("SPAN"), 0.0, None, OP.is_gt)
                    segreduce(t("m_"), "EC")
                    V.tensor_tensor(e8("ED"), e8("TLO"), e8("THI"), OP.add)
                    V.tensor_tensor(e8("THI"), e8("ED"), e8("D8X"), OP.mult)
                    V.scalar_tensor_tensor(e8("THI"), e8("K8X"), 2.0,
                                           e8("THI"), OP.mult, OP.add)
                    V.tensor_tensor(e8("THI"), e8("THI"), e8("EC"), OP.mult)
                    segreduce(t("sx"), "THI")
                    V.tensor_tensor(e8("THI"), e8("ED"), e8("D8Y"), OP.mult)
                    V.scalar_tensor_tensor(e8("THI"), e8("K8Y"), 2.0,
                                           e8("THI"), OP.mult, OP.add)
                    V.tensor_tensor(e8("THI"), e8("THI"), e8("EC"), OP.mult)
                    segreduce(t("sy"), "THI")
                    V.tensor_scalar(t("i2m"), t("m_"), 2.0, 1.0, OP.mult,
                                    OP.max)
                    V.reciprocal(t("i2m"), t("i2m"))
                    V.tensor_tensor(t("cx"), t("sx"), t("i2m"), OP.mult)
                    V.tensor_tensor(t("cy"), t("sy"), t("i2m"), OP.mult)

                    # ---- global-y of starts/dirs (KGY->EC, DGY->ED) --------
                    V.tensor_tensor(sl("ED", 2), w1, s1A, OP.mult)   # wsg
                    V.tensor_scalar(sl("ED", 0), sl("ED", 2), -1.0, None, OP.mult)
                    V.tensor_tensor(sl("ED", 3), h1, c1A, OP.mult)   # hcg
                    V.tensor_scalar(sl("ED", 1), sl("ED", 3), -1.0, None, OP.mult)
                    V.tensor_tensor(sl("ED", 6), w2, s2A, OP.mult)   # vsg
                    V.tensor_scalar(sl("ED", 4), sl("ED", 6), -1.0, None, OP.mult)
                    V.tensor_tensor(sl("ED", 7), h2, c2A, OP.mult)   # gcg
                    V.tensor_scalar(sl("ED", 5), sl("ED", 7), -1.0, None, OP.mult)
                    # S combos reuse E1f..F2f
                    V.tensor_tensor(t("E1f"), sl("ED", 2), sl("ED", 3), OP.add)
                    V.tensor_tensor(t("E2f"), sl("ED", 3), sl("ED", 2), OP.subtract)
                    V.tensor_tensor(t("F1f"), sl("ED", 6), sl("ED", 7), OP.add)
                    V.tensor_tensor(t("F2f"), sl("ED", 7), sl("ED", 6), OP.subtract)
                    V.scalar_tensor_tensor(sl("EC", 0), t("E1f"), 0.5, t("py"), OP.mult, OP.subtract)
                    V.scalar_tensor_tensor(sl("EC", 1), t("E2f"), 0.5, t("py"), OP.mult, OP.subtract)
                    V.scalar_tensor_tensor(sl("EC", 2), t("E1f"), -0.5, t("py"), OP.mult, OP.subtract)
                    V.scalar_tensor_tensor(sl("EC", 3), t("E2f"), -0.5, t("py"), OP.mult, OP.subtract)
                    V.tensor_scalar(sl("EC", 4), t("F1f"), 0.5, None, OP.mult)
                    V.tensor_scalar(sl("EC", 5), t("F2f"), 0.5, None, OP.mult)
                    V.tensor_scalar(sl("EC", 6), t("F1f"), -0.5, None, OP.mult)
                    V.tensor_scalar(sl("EC", 7), t("F2f"), -0.5, None, OP.mult)
                    # centroid global-y
                    V.tensor_tensor(t("tmpA"), s2A, t("cx"), OP.mult)
                    V.tensor_tensor(t("tmpB"), c2A, t("cy"), OP.mult)
                    V.tensor_tensor(t("cgy"), t("tmpA"), t("tmpB"), OP.add)

                    # ---- tau tests + correction ----------------------------
                    # TAU_A -> THI, KGYR -> EA (per slice), TAU_B -> TLO
                    def b8(ft):
                        return ft.rearrange("p (o f) -> p o f", o=1)\
                                 .to_broadcast((P, 8, F))

                    def v8(nm):
                        return E8[nm][:].rearrange("p (s f) -> p s f", s=8)

                    V.tensor_tensor(e8("THI"), e8("TLO"), e8("ED"), OP.mult)
                    V.tensor_tensor(v8("EA"), v8("EC"), b8(t("cgy")),
                                    OP.subtract)
                    V.tensor_tensor(e8("THI"), e8("THI"), e8("EA"), OP.add)
                    V.tensor_tensor(e8("TLO"), e8("SPAN"), e8("ED"), OP.mult)
                    V.tensor_tensor(e8("TLO"), e8("TLO"), e8("THI"), OP.add)
                    V.tensor_scalar(e8("EC"), e8("THI"), 0.0, None, OP.is_ge)
                    V.tensor_scalar(e8("THI"), e8("TLO"), 0.0, None, OP.is_lt)
                    V.tensor_tensor(e8("EC"), e8("EC"), e8("THI"), OP.mult)
                    # chi = cross(c, d): EA = cx*D8Y, THI = cy*D8X (per slice)
                    V.tensor_tensor(v8("EA"), b8(t("cx")), v8("D8Y"),
                                    OP.mult)
                    V.tensor_tensor(v8("THI"), b8(t("cy")), v8("D8X"),
                                    OP.mult)
                    V.tensor_tensor(e8("EA"), e8("EA"), e8("THI"), OP.subtract)
                    V.tensor_tensor(e8("EA"), e8("SPAN"), e8("EA"), OP.mult)
                    V.tensor_tensor(e8("EA"), e8("EB"), e8("EA"), OP.subtract)
                    V.tensor_tensor(e8("EA"), e8("EC"), e8("EA"), OP.mult)

                    segreduce(t("S_"), "EB")
                    segreduce(t("CR_"), "EA")
                    V.tensor_tensor(t("S_"), t("S_"), t("CR_"), OP.subtract)

                    # ---- iou / loss ----------------------------------------
                    V.tensor_scalar(t("inter"), t("S_"), 0.5, 0.0, OP.mult,
                                    OP.max)
                    V.tensor_tensor(t("un"), t("ar1"), t("ar2"), OP.add)
                    V.tensor_tensor(t("un"), t("un"), t("inter"), OP.subtract)
                    V.tensor_scalar(t("un"), t("un"), ALPHA_EPS, None, OP.max)
                    V.reciprocal(t("un"), t("un"))
                    V.tensor_tensor(t("iou"), t("inter"), t("un"), OP.mult)
                    V.tensor_scalar(t("iou"), t("iou"), ALPHA_EPS, None, OP.max)
                    V.tensor_tensor(t("io2"), t("iou"), t("iou"), OP.mult)
                    V.tensor_tensor(t("io3"), t("io2"), t("iou"), OP.mult)
                    V.tensor_tensor(t("junk"), t("io3"), wt[:], OP.mult)
                    V.tensor_reduce(acc4[:, ch:ch + 1], t("junk"), AX.X, OP.add)
                    if ch == NCHUNK - 1:
                        V.drain().then_inc(done_sem, 1)

    return nc


def _get_graph():
    if "nc" not in _cache:
        _cache["nc"] = _build_graph()
    return _cache["nc"]


def _shard_inputs(pred, target, weight):
    """Pad to NCORES*NCORE boxes and split per core."""
    per = NSHARD
    pads = NCORE - per
    pad_box = np.zeros((pads, 5), np.float32)
    pad_box[:, 2] = 1.0
    pad_box[:, 3] = 1.0
    pad_box[:, 4] = 0.3
    in_maps = []
    for c in range(NCORES):
        lo, hi = c * per, (c + 1) * per
        p = np.concatenate([np.ascontiguousarray(pred[lo:hi]), pad_box], 0)
        t = np.concatenate([np.ascontiguousarray(target[lo:hi]), pad_box], 0)
        w = np.concatenate([np.ascontiguousarray(weight[lo:hi]),
                            np.zeros(pads, np.float32)], 0)
        in_maps.append({"pred": p, "target": t, "weight": w})
    return in_maps


def kernel(pred, target, weight):
    from concourse.bass_utils import run_bass_kernel_spmd

    pred = np.asarray(pred, np.float32)
    target = np.asarray(target, np.float32)
    weight = np.asarray(weight, np.float32)

    nc = _get_graph()
    in_maps = _shard_inputs(pred, target, weight)
    res = run_bass_kernel_spmd(nc, in_maps, list(range(NCORES)))
    _cache["last_result"] = res
    total = sum(float(r["out"].astype(np.float64).sum()) for r in res.results)
    wsum = float(weight.astype(np.float64).sum())
    loss = (wsum - total) / NGLOB
    return np.float32(loss)



# revision 6
# speedup vs baseline: 1.3843x; 1.3843x over previous
"""AlphaRotatedIoULoss on 8 TRN2 NeuronCores (raw Bass SPMD kernel, v2).

Sort-free replication of the reference's rotated-IoU loss:
  - intersection area via directed-segment shoelace: clip each box's 4 edges
    against the other box (branch-free Liang-Barsky in that box's local
    frame), then sum span*cross(k,d) over the 8 directed boundary segments
    (all expressed in box2's frame).
  - the reference's shoelace drops the closing edge (last->first angle-sorted
    vertex) because invalid candidate slots are zeroed; the missing term is
    the cross of the unique boundary segment crossing the global -x ray from
    the vertex centroid.  Replicated branch-free via a global-y sign test.
Data-parallel over boxes: each core takes 1/8th, emits per-partition
per-chunk partial sums of weight*iou^3; host:  loss = (sum(w) - total) / n.

v2: F=248 (2 chunks) with manually aliased SBUF scratch, scalings folded
into scalar_tensor_tensor ops, product negations written straight into the
segment-direction slices, analytic cross terms for box2's own (axis-aligned)
edges.
"""
import numpy as np

P = 128          # partitions
T = 496          # boxes per partition per core
F = 248          # chunk width (free dim)
NCHUNK = T // F  # 2
NCORE = P * T    # 63488 boxes per core
NSHARD = 62500   # real boxes per core
NGLOB = 500000
NCORES = 8
ALPHA_EPS = 1e-6
TINY = 1e-12

_cache = {}


def _build_graph():
    import concourse.bass as bass
    import concourse.mybir as mybir
    from contextlib import ExitStack

    dt = mybir.dt.float32
    dt16 = mybir.dt.bfloat16
    AF = mybir.ActivationFunctionType
    OP = mybir.AluOpType
    AX = mybir.AxisListType
    HALF_PI = float(np.pi / 2)

    nc = bass.Bass(detect_race_conditions=False)

    def reg_const(value):
        t = nc.alloc_sbuf_tensor(f"const-f32-{value}", [128, 1], dt)
        nc.gpsimd.memset(t.ap(), value)
        nc.const_aps.aps[(dt, float(value))] = t.ap()

    reg_const(HALF_PI)
    nc.all_engine_barrier()

    pred = nc.declare_dram_parameter("pred", [NCORE, 5], dt, isOutput=False)
    targ = nc.declare_dram_parameter("target", [NCORE, 5], dt, isOutput=False)
    wgt = nc.declare_dram_parameter("weight", [NCORE], dt, isOutput=False)
    out = nc.declare_dram_parameter("out", [P, NCHUNK], dt, isOutput=True)

    predv = pred.rearrange("(p t) f -> p t f", p=P)
    targv = targ.rearrange("(p t) f -> p t f", p=P)
    wv = wgt.rearrange("(p t) -> p t", p=P)

    V = nc.vector
    A = nc.scalar

    with ExitStack() as ctx:
        _n = [0]

        def alloc(shape, dtype=dt):
            _n[0] += 1
            return ctx.enter_context(
                nc.sbuf_tensor(f"tile{_n[0]}", shape, dtype))

        # double-buffered inputs (2 chunks -> one buffer each, no recycling)
        pt2 = [alloc([P, F, 5]) for _ in range(2)]
        tg2 = [alloc([P, F, 5]) for _ in range(2)]
        wt2 = [alloc([P, F]) for _ in range(2)]
        # ACT-owned trig outputs (bf16: feed DVE 2x-mode tensor ops)
        sdr_t, cd_t, s1_t, c1_t, s2_t, c2_t = (
            alloc([P, F], dt16) for _ in range(6))
        ltmp = [alloc([P, F]) for _ in range(4)]  # fp32: ln/exp log-domain
        # F-sized DVE scratch.  Geometry-magnitude tiles go bf16 (2x/4x DVE
        # modes); fp32 for trig args, reduce outputs, and the loss tail.
        FN32 = ("delta tmpA tmpB px py ar1 ar2 "
                "m_ sx sy i2m cx cy S_ CR_ inter un iou io2 io3 junk").split()
        FN16 = ("W1 H1 W2 H2 ar2h sg asd sd "
                "vcf vsf gcf gsf mu mv nu nv E1f E2f F1f F2f P1f P2f Q1f Q2f "
                "awsf ahsf avsf agsf cx16 cy16 cgy16 tA16 tB16").split()
        FT = {n: alloc([P, F]) for n in FN32}
        FT.update({n: alloc([P, F], dt16) for n in FN16})
        # 4F tiles
        C4 = {n: alloc([P, 4 * F], dt16) for n in
              "KXP KXM KYP KYM RXA RYA RXB RYB GX4 GY4".split()}
        # 8F tiles
        E8 = {n: alloc([P, 8 * F], dt16) for n in
              "K8X K8Y D8X D8Y TLO THI SPAN EA EB EC ED".split()}
        acc4 = alloc([P, NCHUNK])

        with (
            nc.semaphore("dma_sem") as dma_sem,
            nc.semaphore("v_sem") as v_sem,
            nc.semaphore("a_sem") as a_sem,
            nc.semaphore("v2_sem") as v2_sem,
            nc.semaphore("a2_sem") as a2_sem,
            nc.semaphore("done_sem") as done_sem,
            nc.Block() as block,
        ):
            @block.sync
            def _(sync):
                for ch in range(NCHUNK):
                    sync.dma_start(
                        out=pt2[ch][:], in_=predv[:, ch * F:(ch + 1) * F, :]
                    ).then_inc(dma_sem, 16)
                    sync.dma_start(
                        out=tg2[ch][:], in_=targv[:, ch * F:(ch + 1) * F, :]
                    ).then_inc(dma_sem, 16)
                    sync.dma_start(
                        out=wt2[ch][:], in_=wv[:, ch * F:(ch + 1) * F]
                    ).then_inc(dma_sem, 16)
                sync.wait_ge(done_sem, 1)
                sync.dma_start(out=out[:], in_=acc4[:]).then_inc(dma_sem, 16)

            @block.scalar
            def _(scalar):
                for ch in range(NCHUNK):
                    a1 = pt2[ch][:, :, 4]
                    a2 = tg2[ch][:, :, 4]
                    scalar.wait_ge(v_sem, ch + 1)
                    A.activation(sdr_t[:], FT["delta"][:], AF.Sin)
                    A.activation(cd_t[:], FT["delta"][:], AF.Sin, bias=HALF_PI)
                    A.activation(s1_t[:], a1, AF.Sin)
                    A.activation(c1_t[:], a1, AF.Sin, bias=HALF_PI)
                    A.activation(s2_t[:], a2, AF.Sin)
                    A.activation(c2_t[:], FT["tmpA"][:], AF.Sin)
                    A.drain().then_inc(a_sem, 1)
                    scalar.wait_ge(v2_sem, ch + 1)
                    # clip reciprocals: r = exp(-ln(x)), x > 0
                    wcf_s = E8["D8X"][:, 2 * F:3 * F]
                    hcf_s = E8["D8Y"][:, 3 * F:4 * F]
                    for ins, outs in (
                        ((wcf_s, FT["ahsf"][:], FT["awsf"][:], hcf_s),
                         (C4["RXA"][:, 0:F], C4["RXA"][:, 3 * F:4 * F],
                          C4["RYA"][:, 0:F], C4["RYA"][:, F:2 * F])),
                        ((FT["vcf"][:], FT["agsf"][:], FT["avsf"][:],
                          FT["gcf"][:]),
                         (C4["RXB"][:, 0:F], C4["RXB"][:, F:2 * F],
                          C4["RYB"][:, 2 * F:3 * F], C4["RYB"][:, F:2 * F])),
                    ):
                        for j in range(4):
                            A.activation(ltmp[j][:], ins[j], AF.Ln)
                        for j in range(4):
                            A.activation(outs[j], ltmp[j][:], AF.Exp,
                                         scale=-1.0)
                    A.drain().then_inc(a2_sem, 1)

            @block.vector
            def _(vector):
                t = lambda n: FT[n][:]
                c4 = lambda n: C4[n][:]
                e8 = lambda n: E8[n][:]

                def sl(nm, i):
                    return E8[nm][:, i * F:(i + 1) * F]

                def sl4(nm, i):
                    return C4[nm][:, i * F:(i + 1) * F]

                def segreduce(dst, nm):
                    v = E8[nm][:].rearrange("p (s f) -> p f s", s=8)
                    V.tensor_reduce(dst, v, AX.X, OP.add)

                # constant zero slices of D8X/D8Y (box2's own AA edge dirs)
                V.memset(sl("D8X", 5), 0.0)
                V.memset(sl("D8X", 7), 0.0)
                V.memset(sl("D8Y", 4), 0.0)
                V.memset(sl("D8Y", 6), 0.0)

                for ch in range(NCHUNK):
                    pt, tg, wt = pt2[ch], tg2[ch], wt2[ch]
                    x1, y1, w1, h1, a1 = (pt[:, :, i] for i in range(5))
                    x2, y2, w2, h2, a2 = (tg[:, :, i] for i in range(5))

                    vector.wait_ge(dma_sem, 48 * (ch + 1))
                    # angles for ACT: delta, and wrapped a2+pi/2 in tmpA
                    V.tensor_tensor(t("delta"), a1, a2, OP.subtract)
                    V.tensor_scalar(t("tmpA"), a2, HALF_PI, None, OP.add)
                    V.tensor_scalar(t("tmpB"), t("tmpA"), float(np.pi), None,
                                    OP.is_gt)
                    V.scalar_tensor_tensor(t("tmpA"), t("tmpB"),
                                           float(-2 * np.pi), t("tmpA"),
                                           OP.mult, OP.add)
                    V.drain().then_inc(v_sem, 1)

                    # trig-independent work
                    V.tensor_tensor(t("px"), x2, x1, OP.subtract)
                    V.tensor_tensor(t("py"), y2, y1, OP.subtract)
                    V.tensor_scalar(t("W1"), w1, 0.5, None, OP.mult)
                    V.tensor_scalar(t("H1"), h1, 0.5, None, OP.mult)
                    V.tensor_scalar(t("W2"), w2, 0.5, None, OP.mult)
                    V.tensor_scalar(t("H2"), h2, 0.5, None, OP.mult)
                    V.tensor_tensor(t("ar1"), w1, h1, OP.mult)
                    V.tensor_tensor(t("ar2"), w2, h2, OP.mult)
                    V.tensor_scalar(t("ar2h"), t("ar2"), 0.5, None, OP.mult)
                    # box2 own corners -> K8X/K8Y slices 4..7
                    V.tensor_copy(sl("K8X", 4), t("W2"))
                    V.tensor_scalar(sl("K8X", 5), t("W2"), -1.0, None, OP.mult)
                    V.tensor_copy(sl("K8X", 6), sl("K8X", 5))
                    V.tensor_copy(sl("K8X", 7), t("W2"))
                    V.tensor_copy(sl("K8Y", 4), t("H2"))
                    V.tensor_copy(sl("K8Y", 5), t("H2"))
                    V.tensor_scalar(sl("K8Y", 6), t("H2"), -1.0, None, OP.mult)
                    V.tensor_copy(sl("K8Y", 7), sl("K8Y", 6))
                    # box2 own edge dirs -> D8X/D8Y slices 4..7 (x: -w2,0,w2,0)
                    V.tensor_scalar(sl("D8X", 4), w2, -1.0, None, OP.mult)
                    V.tensor_scalar(sl("D8X", 6), sl("D8X", 4), -1.0, None,
                                    OP.mult)
                    V.tensor_scalar(sl("D8Y", 5), h2, -1.0, None, OP.mult)
                    V.tensor_scalar(sl("D8Y", 7), sl("D8Y", 5), -1.0, None,
                                    OP.mult)

                    # ---- trig-dependent ------------------------------------
                    vector.wait_ge(a_sem, ch + 1)
                    cdA, s1A, c1A, s2A, c2A = (cd_t[:], s1_t[:], c1_t[:],
                                               s2_t[:], c2_t[:])
                    V.tensor_scalar(t("sg"), sdr_t[:], 0.0, None, OP.is_ge)
                    V.tensor_scalar(t("sg"), t("sg"), 2.0, -1.0, OP.mult,
                                    OP.add)
                    V.tensor_tensor(t("asd"), t("sg"), sdr_t[:], OP.mult)
                    V.tensor_scalar(t("asd"), t("asd"), TINY, None, OP.max)
                    V.tensor_tensor(t("sd"), t("sg"), t("asd"), OP.mult)
                    sdA = t("sd")

                    # box1 full products straight into D8 slices 0..3
                    wcf = sl("D8X", 2); V.tensor_tensor(wcf, w1, cdA, OP.mult)
                    V.tensor_scalar(sl("D8X", 0), wcf, -1.0, None, OP.mult)
                    wsf = sl("D8Y", 2); V.tensor_tensor(wsf, w1, sdA, OP.mult)
                    V.tensor_scalar(sl("D8Y", 0), wsf, -1.0, None, OP.mult)
                    hsf = sl("D8X", 1); V.tensor_tensor(hsf, h1, sdA, OP.mult)
                    V.tensor_scalar(sl("D8X", 3), hsf, -1.0, None, OP.mult)
                    hcf = sl("D8Y", 3); V.tensor_tensor(hcf, h1, cdA, OP.mult)
                    V.tensor_scalar(sl("D8Y", 1), hcf, -1.0, None, OP.mult)
                    # box2 full products (kept as F tiles; clip-B recips)
                    V.tensor_tensor(t("vcf"), w2, cdA, OP.mult)
                    V.tensor_tensor(t("vsf"), w2, sdA, OP.mult)
                    V.tensor_tensor(t("gcf"), h2, cdA, OP.mult)
                    V.tensor_tensor(t("gsf"), h2, sdA, OP.mult)
                    # |.| products for the ACT reciprocals of signed inputs
                    V.tensor_tensor(t("awsf"), w1, t("asd"), OP.mult)
                    V.tensor_tensor(t("ahsf"), h1, t("asd"), OP.mult)
                    V.tensor_tensor(t("avsf"), w2, t("asd"), OP.mult)
                    V.tensor_tensor(t("agsf"), h2, t("asd"), OP.mult)
                    V.drain().then_inc(v2_sem, 1)

                    # centers
                    V.tensor_tensor(t("tmpA"), t("px"), c2A, OP.mult)
                    V.tensor_tensor(t("tmpB"), t("py"), s2A, OP.mult)
                    V.scalar_tensor_tensor(t("mu"), t("tmpA"), -1.0, t("tmpB"),
                                           OP.mult, OP.subtract)
                    V.tensor_tensor(t("tmpA"), t("px"), s2A, OP.mult)
                    V.tensor_tensor(t("tmpB"), t("py"), c2A, OP.mult)
                    V.tensor_tensor(t("mv"), t("tmpA"), t("tmpB"), OP.subtract)
                    V.tensor_tensor(t("tmpA"), t("px"), c1A, OP.mult)
                    V.tensor_tensor(t("tmpB"), t("py"), s1A, OP.mult)
                    V.tensor_tensor(t("nu"), t("tmpA"), t("tmpB"), OP.add)
                    V.tensor_tensor(t("tmpA"), t("px"), s1A, OP.mult)
                    V.tensor_tensor(t("tmpB"), t("py"), c1A, OP.mult)
                    V.tensor_tensor(t("nv"), t("tmpB"), t("tmpA"), OP.subtract)

                    # box1 corners in box2 frame (half-scales folded into STT)
                    V.tensor_tensor(t("E1f"), wcf, hsf, OP.subtract)
                    V.tensor_tensor(t("E2f"), wcf, hsf, OP.add)
                    V.tensor_tensor(t("F1f"), wsf, hcf, OP.add)
                    V.tensor_tensor(t("F2f"), wsf, hcf, OP.subtract)
                    V.scalar_tensor_tensor(sl("K8X", 0), t("E1f"), 0.5, t("mu"), OP.mult, OP.add)
                    V.scalar_tensor_tensor(sl("K8X", 1), t("E2f"), -0.5, t("mu"), OP.mult, OP.add)
                    V.scalar_tensor_tensor(sl("K8X", 2), t("E1f"), -0.5, t("mu"), OP.mult, OP.add)
                    V.scalar_tensor_tensor(sl("K8X", 3), t("E2f"), 0.5, t("mu"), OP.mult, OP.add)
                    V.scalar_tensor_tensor(sl("K8Y", 0), t("F1f"), 0.5, t("mv"), OP.mult, OP.add)
                    V.scalar_tensor_tensor(sl("K8Y", 1), t("F2f"), -0.5, t("mv"), OP.mult, OP.add)
                    V.scalar_tensor_tensor(sl("K8Y", 2), t("F1f"), -0.5, t("mv"), OP.mult, OP.add)
                    V.scalar_tensor_tensor(sl("K8Y", 3), t("F2f"), 0.5, t("mv"), OP.mult, OP.add)
                    # box2 corners in box1 frame
                    V.tensor_tensor(t("P1f"), t("vcf"), t("gsf"), OP.add)
                    V.tensor_tensor(t("P2f"), t("vcf"), t("gsf"), OP.subtract)
                    V.tensor_tensor(t("Q1f"), t("gcf"), t("vsf"), OP.subtract)
                    V.tensor_tensor(t("Q2f"), t("gcf"), t("vsf"), OP.add)
                    V.scalar_tensor_tensor(sl4("GX4", 0), t("P1f"), 0.5, t("nu"), OP.mult, OP.add)
                    V.scalar_tensor_tensor(sl4("GX4", 1), t("P2f"), -0.5, t("nu"), OP.mult, OP.add)
                    V.scalar_tensor_tensor(sl4("GX4", 2), t("P1f"), -0.5, t("nu"), OP.mult, OP.add)
                    V.scalar_tensor_tensor(sl4("GX4", 3), t("P2f"), 0.5, t("nu"), OP.mult, OP.add)
                    V.scalar_tensor_tensor(sl4("GY4", 0), t("Q1f"), 0.5, t("nv"), OP.mult, OP.add)
                    V.scalar_tensor_tensor(sl4("GY4", 1), t("Q2f"), 0.5, t("nv"), OP.mult, OP.add)
                    V.scalar_tensor_tensor(sl4("GY4", 2), t("Q1f"), -0.5, t("nv"), OP.mult, OP.add)
                    V.scalar_tensor_tensor(sl4("GY4", 3), t("Q2f"), -0.5, t("nv"), OP.mult, OP.add)

                    def b3(ft):
                        # [P,F] -> broadcast [P,4,F]
                        return ft.rearrange("p (o f) -> p o f", o=1)\
                                 .to_broadcast((P, 4, F))

                    def v3(ap4):
                        return ap4.rearrange("p (s f) -> p s f", s=4)

                    vector.wait_ge(a2_sem, ch + 1)

                    def emit_clip(corner_x4, corner_y4, Wb, Hb,
                                  RXn, RYn, sgn_slots,
                                  lo_out, hi_out, span_out):
                        # shifted corners, batched with broadcast bounds
                        V.tensor_tensor(v3(c4("KXP")), v3(corner_x4), b3(Wb),
                                        OP.add)
                        V.tensor_tensor(v3(c4("KXM")), v3(corner_x4), b3(Wb),
                                        OP.subtract)
                        V.tensor_tensor(v3(c4("KYP")), v3(corner_y4), b3(Hb),
                                        OP.add)
                        V.tensor_tensor(v3(c4("KYM")), v3(corner_y4), b3(Hb),
                                        OP.subtract)
                        # R tiles: ACT prefilled positive slices; apply sign
                        # to the signed ones in place, then fill negated slots
                        for Rt, (pos0, neg0, pos1, neg1, signed) in (
                                (RXn, sgn_slots[0]), (RYn, sgn_slots[1])):
                            for s in signed:
                                V.tensor_tensor(sl4(Rt, s), t("sg"),
                                                sl4(Rt, s), OP.mult)
                            V.tensor_scalar(sl4(Rt, neg0), sl4(Rt, pos0),
                                            -1.0, None, OP.mult)
                            V.tensor_scalar(sl4(Rt, neg1), sl4(Rt, pos1),
                                            -1.0, None, OP.mult)
                        V.tensor_tensor(c4("KXP"), c4("KXP"), c4(RXn), OP.mult)
                        V.tensor_tensor(c4("KXM"), c4("KXM"), c4(RXn), OP.mult)
                        V.tensor_tensor(c4("KYP"), c4("KYP"), c4(RYn), OP.mult)
                        V.tensor_tensor(c4("KYM"), c4("KYM"), c4(RYn), OP.mult)
                        V.tensor_tensor(c4(RXn), c4("KXP"), c4("KXM"), OP.min)
                        V.tensor_tensor(c4("KXP"), c4("KXP"), c4("KXM"), OP.max)
                        V.tensor_tensor(c4(RYn), c4("KYP"), c4("KYM"), OP.min)
                        V.tensor_tensor(c4("KYP"), c4("KYP"), c4("KYM"), OP.max)
                        V.tensor_tensor(lo_out, c4(RXn), c4(RYn), OP.max)
                        V.tensor_scalar(lo_out, lo_out, 0.0, None, OP.max)
                        V.tensor_tensor(hi_out, c4("KXP"), c4("KYP"), OP.min)
                        V.tensor_scalar(hi_out, hi_out, 1.0, None, OP.min)
                        V.scalar_tensor_tensor(span_out, lo_out, -1.0, hi_out,
                                               OP.mult, OP.add)
                        V.tensor_scalar(span_out, span_out, 0.0, None, OP.max)

                    # part A: box1 edges vs AA box2
                    # RXA: pos0=0(wcf,+) neg->2; pos1=3(|hsf|,signed) neg->1
                    # RYA: pos0=0(|wsf|,signed) neg->2; pos1=1(hcf,+) neg->3
                    emit_clip(E8["K8X"][:, 0:4 * F], E8["K8Y"][:, 0:4 * F],
                              t("W2"), t("H2"), "RXA", "RYA",
                              ((0, 2, 3, 1, (3,)), (0, 2, 1, 3, (0,))),
                              E8["TLO"][:, 0:4 * F], E8["THI"][:, 0:4 * F],
                              E8["SPAN"][:, 0:4 * F])
                    # part B: box2 edges vs AA box1
                    # RXB: pos0=0(vcf,+) neg->2; pos1=1(|gsf|,signed) neg->3
                    # RYB: pos0=2(|vsf|,signed) neg->0; pos1=1(gcf,+) neg->3
                    emit_clip(c4("GX4"), c4("GY4"),
                              t("W1"), t("H1"), "RXB", "RYB",
                              ((0, 2, 1, 3, (1,)), (2, 0, 1, 3, (2,))),
                              E8["TLO"][:, 4 * F:8 * F],
                              E8["THI"][:, 4 * F:8 * F],
                              E8["SPAN"][:, 4 * F:8 * F])

                    # ---- area terms ----------------------------------------
                    # A half: numeric cross(k,d); B half: cross = 2*W2*H2
                    EAa = E8["EA"][:, 0:4 * F]
                    EBa = E8["EB"][:, 0:4 * F]
                    V.tensor_tensor(EAa, E8["K8X"][:, 0:4 * F],
                                    E8["D8Y"][:, 0:4 * F], OP.mult)
                    V.tensor_tensor(EBa, E8["K8Y"][:, 0:4 * F],
                                    E8["D8X"][:, 0:4 * F], OP.mult)
                    V.tensor_tensor(EAa, EAa, EBa, OP.subtract)
                    V.tensor_tensor(EBa, E8["SPAN"][:, 0:4 * F], EAa, OP.mult)
                    V.tensor_tensor(
                        E8["EB"][:, 4 * F:8 * F].rearrange("p (s f) -> p s f", s=4),
                        E8["SPAN"][:, 4 * F:8 * F].rearrange("p (s f) -> p s f", s=4),
                        b3(t("ar2h")), OP.mult)
                    # EB = SV8 (span * cross) for all 8 segments

                    # ---- vertex centroid -----------------------------------
                    V.tensor_scalar(e8("EC"), e8("SPAN"), 0.0, None, OP.is_gt)
                    segreduce(t("m_"), "EC")
                    V.tensor_tensor(e8("ED"), e8("TLO"), e8("THI"), OP.add)
                    V.tensor_tensor(e8("THI"), e8("ED"), e8("D8X"), OP.mult)
                    V.scalar_tensor_tensor(e8("THI"), e8("K8X"), 2.0,
                                           e8("THI"), OP.mult, OP.add)
                    V.tensor_tensor(e8("THI"), e8("THI"), e8("EC"), OP.mult)
                    segreduce(t("sx"), "THI")
                    V.tensor_tensor(e8("THI"), e8("ED"), e8("D8Y"), OP.mult)
                    V.scalar_tensor_tensor(e8("THI"), e8("K8Y"), 2.0,
                                           e8("THI"), OP.mult, OP.add)
                    V.tensor_tensor(e8("THI"), e8("THI"), e8("EC"), OP.mult)
                    segreduce(t("sy"), "THI")
                    V.tensor_scalar(t("i2m"), t("m_"), 2.0, 1.0, OP.mult,
                                    OP.max)
                    V.reciprocal(t("i2m"), t("i2m"))
                    V.tensor_tensor(t("cx"), t("sx"), t("i2m"), OP.mult)
                    V.tensor_tensor(t("cy"), t("sy"), t("i2m"), OP.mult)

                    # ---- global-y of starts/dirs (KGY->EC, DGY->ED) --------
                    V.tensor_tensor(sl("ED", 2), w1, s1A, OP.mult)   # wsg
                    V.tensor_scalar(sl("ED", 0), sl("ED", 2), -1.0, None, OP.mult)
                    V.tensor_tensor(sl("ED", 3), h1, c1A, OP.mult)   # hcg
                    V.tensor_scalar(sl("ED", 1), sl("ED", 3), -1.0, None, OP.mult)
                    V.tensor_tensor(sl("ED", 6), w2, s2A, OP.mult)   # vsg
                    V.tensor_scalar(sl("ED", 4), sl("ED", 6), -1.0, None, OP.mult)
                    V.tensor_tensor(sl("ED", 7), h2, c2A, OP.mult)   # gcg
                    V.tensor_scalar(sl("ED", 5), sl("ED", 7), -1.0, None, OP.mult)
                    # S combos reuse E1f..F2f
                    V.tensor_tensor(t("E1f"), sl("ED", 2), sl("ED", 3), OP.add)
                    V.tensor_tensor(t("E2f"), sl("ED", 3), sl("ED", 2), OP.subtract)
                    V.tensor_tensor(t("F1f"), sl("ED", 6), sl("ED", 7), OP.add)
                    V.tensor_tensor(t("F2f"), sl("ED", 7), sl("ED", 6), OP.subtract)
                    V.scalar_tensor_tensor(sl("EC", 0), t("E1f"), 0.5, t("py"), OP.mult, OP.subtract)
                    V.scalar_tensor_tensor(sl("EC", 1), t("E2f"), 0.5, t("py"), OP.mult, OP.subtract)
                    V.scalar_tensor_tensor(sl("EC", 2), t("E1f"), -0.5, t("py"), OP.mult, OP.subtract)
                    V.scalar_tensor_tensor(sl("EC", 3), t("E2f"), -0.5, t("py"), OP.mult, OP.subtract)
                    V.tensor_scalar(sl("EC", 4), t("F1f"), 0.5, None, OP.mult)
                    V.tensor_scalar(sl("EC", 5), t("F2f"), 0.5, None, OP.mult)
                    V.tensor_scalar(sl("EC", 6), t("F1f"), -0.5, None, OP.mult)
                    V.tensor_scalar(sl("EC", 7), t("F2f"), -0.5, None, OP.mult)
                    # centroid global-y (bf16 copies feed the 8F broadcasts)
                    V.tensor_copy(t("cx16"), t("cx"))
                    V.tensor_copy(t("cy16"), t("cy"))
                    V.tensor_tensor(t("tA16"), s2A, t("cx16"), OP.mult)
                    V.tensor_tensor(t("tB16"), c2A, t("cy16"), OP.mult)
                    V.tensor_tensor(t("cgy16"), t("tA16"), t("tB16"), OP.add)

                    # ---- tau tests + correction ----------------------------
                    # TAU_A -> THI, KGYR -> EA (per slice), TAU_B -> TLO
                    def b8(ft):
                        return ft.rearrange("p (o f) -> p o f", o=1)\
                                 .to_broadcast((P, 8, F))

                    def v8(nm):
                        return E8[nm][:].rearrange("p (s f) -> p s f", s=8)

                    V.tensor_tensor(e8("THI"), e8("TLO"), e8("ED"), OP.mult)
                    V.tensor_tensor(v8("EA"), v8("EC"), b8(t("cgy16")),
                                    OP.subtract)
                    V.tensor_tensor(e8("THI"), e8("THI"), e8("EA"), OP.add)
                    V.tensor_tensor(e8("TLO"), e8("SPAN"), e8("ED"), OP.mult)
                    V.tensor_tensor(e8("TLO"), e8("TLO"), e8("THI"), OP.add)
                    V.tensor_scalar(e8("EC"), e8("THI"), 0.0, None, OP.is_ge)
                    V.tensor_scalar(e8("THI"), e8("TLO"), 0.0, None, OP.is_lt)
                    V.tensor_tensor(e8("EC"), e8("EC"), e8("THI"), OP.mult)
                    # chi = cross(c, d): EA = cx*D8Y, THI = cy*D8X (per slice)
                    V.tensor_tensor(v8("EA"), b8(t("cx16")), v8("D8Y"),
                                    OP.mult)
                    V.tensor_tensor(v8("THI"), b8(t("cy16")), v8("D8X"),
                                    OP.mult)
                    V.tensor_tensor(e8("EA"), e8("EA"), e8("THI"), OP.subtract)
                    V.tensor_tensor(e8("EA"), e8("SPAN"), e8("EA"), OP.mult)
                    V.tensor_tensor(e8("EA"), e8("EB"), e8("EA"), OP.subtract)
                    V.tensor_tensor(e8("EA"), e8("EC"), e8("EA"), OP.mult)

                    segreduce(t("S_"), "EB")
                    segreduce(t("CR_"), "EA")
                    V.tensor_tensor(t("S_"), t("S_"), t("CR_"), OP.subtract)

                    # ---- iou / loss ----------------------------------------
                    V.tensor_scalar(t("inter"), t("S_"), 0.5, 0.0, OP.mult,
                                    OP.max)
                    V.tensor_tensor(t("un"), t("ar1"), t("ar2"), OP.add)
                    V.tensor_tensor(t("un"), t("un"), t("inter"), OP.subtract)
                    V.tensor_scalar(t("un"), t("un"), ALPHA_EPS, None, OP.max)
                    V.reciprocal(t("un"), t("un"))
                    V.tensor_tensor(t("iou"), t("inter"), t("un"), OP.mult)
                    V.tensor_scalar(t("iou"), t("iou"), ALPHA_EPS, None, OP.max)
                    V.tensor_tensor(t("io2"), t("iou"), t("iou"), OP.mult)
                    V.tensor_tensor(t("io3"), t("io2"), t("iou"), OP.mult)
                    V.tensor_tensor(t("junk"), t("io3"), wt[:], OP.mult)
                    V.tensor_reduce(acc4[:, ch:ch + 1], t("junk"), AX.X, OP.add)
                    if ch == NCHUNK - 1:
                        V.drain().then_inc(done_sem, 1)

    return nc


def _get_graph():
    if "nc" not in _cache:
        _cache["nc"] = _build_graph()
    return _cache["nc"]


def _shard_inputs(pred, target, weight):
    """Pad to NCORES*NCORE boxes and split per core."""
    per = NSHARD
    pads = NCORE - per
    pad_box = np.zeros((pads, 5), np.float32)
    pad_box[:, 2] = 1.0
    pad_box[:, 3] = 1.0
    pad_box[:, 4] = 0.3
    in_maps = []
    for c in range(NCORES):
        lo, hi = c * per, (c + 1) * per
        p = np.concatenate([np.ascontiguousarray(pred[lo:hi]), pad_box], 0)
        t = np.concatenate([np.ascontiguousarray(target[lo:hi]), pad_box], 0)
        w = np.concatenate([np.ascontiguousarray(weight[lo:hi]),
                            np.zeros(pads, np.float32)], 0)
        in_maps.append({"pred": p, "target": t, "weight": w})
    return in_maps


def kernel(pred, target, weight):
    from concourse.bass_utils import run_bass_kernel_spmd

    pred = np.asarray(pred, np.float32)
    target = np.asarray(target, np.float32)
    weight = np.asarray(weight, np.float32)

    nc = _get_graph()
    in_maps = _shard_inputs(pred, target, weight)
    res = run_bass_kernel_spmd(nc, in_maps, list(range(NCORES)))
    _cache["last_result"] = res
    total = sum(float(r["out"].astype(np.float64).sum()) for r in res.results)
    wsum = float(weight.astype(np.float64).sum())
    loss = (wsum - total) / NGLOB
    return np.float32(loss)



# revision 9
# speedup vs baseline: 1.4178x; 1.0242x over previous
"""AlphaRotatedIoULoss on 8 TRN2 NeuronCores (raw Bass SPMD kernel, v2).

Sort-free replication of the reference's rotated-IoU loss:
  - intersection area via directed-segment shoelace: clip each box's 4 edges
    against the other box (branch-free Liang-Barsky in that box's local
    frame), then sum span*cross(k,d) over the 8 directed boundary segments
    (all expressed in box2's frame).
  - the reference's shoelace drops the closing edge (last->first angle-sorted
    vertex) because invalid candidate slots are zeroed; the missing term is
    the cross of the unique boundary segment crossing the global -x ray from
    the vertex centroid.  Replicated branch-free via a global-y sign test.
Data-parallel over boxes: each core takes 1/8th, emits per-partition
per-chunk partial sums of weight*iou^3; host:  loss = (sum(w) - total) / n.

v2: F=248 (2 chunks) with manually aliased SBUF scratch, scalings folded
into scalar_tensor_tensor ops, product negations written straight into the
segment-direction slices, analytic cross terms for box2's own (axis-aligned)
edges.
"""
import numpy as np

P = 128          # partitions
T = 496          # boxes per partition per core
F = 248          # chunk width (free dim)
NCHUNK = T // F  # 2
NCORE = P * T    # 63488 boxes per core
NSHARD = 62500   # real boxes per core
NGLOB = 500000
NCORES = 8
ALPHA_EPS = 1e-6
TINY = 1e-12

_cache = {}


def _build_graph():
    import concourse.bass as bass
    import concourse.mybir as mybir
    from contextlib import ExitStack

    dt = mybir.dt.float32
    dt16 = mybir.dt.bfloat16
    AF = mybir.ActivationFunctionType
    OP = mybir.AluOpType
    AX = mybir.AxisListType
    HALF_PI = float(np.pi / 2)

    nc = bass.Bass(detect_race_conditions=False)

    def reg_const(value):
        t = nc.alloc_sbuf_tensor(f"const-f32-{value}", [128, 1], dt)
        nc.gpsimd.memset(t.ap(), value)
        nc.const_aps.aps[(dt, float(value))] = t.ap()

    reg_const(HALF_PI)
    nc.all_engine_barrier()

    pred = nc.declare_dram_parameter("pred", [NCORE, 5], dt, isOutput=False)
    targ = nc.declare_dram_parameter("target", [NCORE, 5], dt, isOutput=False)
    wgt = nc.declare_dram_parameter("weight", [NCORE], dt, isOutput=False)
    out = nc.declare_dram_parameter("out", [P, NCHUNK], dt, isOutput=True)

    predv = pred.rearrange("(p t) f -> p t f", p=P)
    targv = targ.rearrange("(p t) f -> p t f", p=P)
    wv = wgt.rearrange("(p t) -> p t", p=P)

    V = nc.vector
    A = nc.scalar

    with ExitStack() as ctx:
        _n = [0]

        def alloc(shape, dtype=dt):
            _n[0] += 1
            return ctx.enter_context(
                nc.sbuf_tensor(f"tile{_n[0]}", shape, dtype))

        # double-buffered inputs (2 chunks -> one buffer each, no recycling)
        pt2 = [alloc([P, F, 5]) for _ in range(2)]
        tg2 = [alloc([P, F, 5]) for _ in range(2)]
        wt2 = [alloc([P, F]) for _ in range(2)]
        # ACT-owned trig outputs (bf16: feed DVE 2x-mode tensor ops)
        sdr_t, cd_t, s1_t, c1_t, s2_t, c2_t = (
            alloc([P, F], dt16) for _ in range(6))
        ltmp = [alloc([P, F]) for _ in range(4)]  # fp32: ln/exp log-domain
        # F-sized DVE scratch.  Geometry-magnitude tiles go bf16 (2x/4x DVE
        # modes); fp32 for trig args, reduce outputs, and the loss tail.
        FN32 = ("delta tmpA tmpB px py ar1 ar2 "
                "m_ sx sy i2m cx cy S_ CR_ inter un iou io2 io3 junk").split()
        FN16 = ("W1 H1 W2 H2 ar2h sg asd sd "
                "vcf vsf gcf gsf mu mv nu nv E1f E2f F1f F2f P1f P2f Q1f Q2f "
                "awsf ahsf avsf agsf cx16 cy16 cgy16 tA16 tB16").split()
        FT = {n: alloc([P, F]) for n in FN32}
        FT.update({n: alloc([P, F], dt16) for n in FN16})
        # 4F tiles
        C4 = {n: alloc([P, 4 * F], dt16) for n in
              "KXP KXM KYP KYM RXA RYA RXB RYB GX4 GY4".split()}
        # 8F tiles
        E8 = {n: alloc([P, 8 * F], dt16) for n in
              "K8X K8Y D8X D8Y TLO THI SPAN EA EB EC ED".split()}
        acc4 = alloc([P, NCHUNK])

        with (
            nc.semaphore("dma_sem") as dma_sem,
            nc.semaphore("v_sem") as v_sem,
            nc.semaphore("a_sem") as a_sem,
            nc.semaphore("v2_sem") as v2_sem,
            nc.semaphore("a2_sem") as a2_sem,
            nc.semaphore("done_sem") as done_sem,
            nc.Block() as block,
        ):
            @block.sync
            def _(sync):
                for ch in range(NCHUNK):
                    sync.dma_start(
                        out=pt2[ch][:], in_=predv[:, ch * F:(ch + 1) * F, :]
                    ).then_inc(dma_sem, 16)
                    sync.dma_start(
                        out=tg2[ch][:], in_=targv[:, ch * F:(ch + 1) * F, :]
                    ).then_inc(dma_sem, 16)
                    sync.dma_start(
                        out=wt2[ch][:], in_=wv[:, ch * F:(ch + 1) * F]
                    ).then_inc(dma_sem, 16)
                sync.wait_ge(done_sem, 1)
                sync.dma_start(out=out[:], in_=acc4[:]).then_inc(dma_sem, 16)

            @block.scalar
            def _(scalar):
                for ch in range(NCHUNK):
                    a1 = pt2[ch][:, :, 4]
                    a2 = tg2[ch][:, :, 4]
                    scalar.wait_ge(v_sem, ch + 1)
                    A.activation(sdr_t[:], FT["delta"][:], AF.Sin)
                    A.activation(cd_t[:], FT["delta"][:], AF.Sin, bias=HALF_PI)
                    A.activation(s1_t[:], a1, AF.Sin)
                    A.activation(c1_t[:], a1, AF.Sin, bias=HALF_PI)
                    A.activation(s2_t[:], a2, AF.Sin)
                    A.activation(c2_t[:], FT["tmpA"][:], AF.Sin)
                    A.drain().then_inc(a_sem, 1)
                    scalar.wait_ge(v2_sem, ch + 1)
                    # clip reciprocals: r = exp(-ln(x)), x > 0
                    wcf_s = E8["D8X"][:, 2 * F:3 * F]
                    hcf_s = E8["D8Y"][:, 3 * F:4 * F]
                    for ins, outs in (
                        ((wcf_s, FT["ahsf"][:], FT["awsf"][:], hcf_s),
                         (C4["RXA"][:, 0:F], C4["RXA"][:, 3 * F:4 * F],
                          C4["RYA"][:, 0:F], C4["RYA"][:, F:2 * F])),
                        ((FT["vcf"][:], FT["agsf"][:], FT["avsf"][:],
                          FT["gcf"][:]),
                         (C4["RXB"][:, 0:F], C4["RXB"][:, F:2 * F],
                          C4["RYB"][:, 2 * F:3 * F], C4["RYB"][:, F:2 * F])),
                    ):
                        for j in range(4):
                            A.activation(ltmp[j][:], ins[j], AF.Ln)
                        for j in range(4):
                            A.activation(outs[j], ltmp[j][:], AF.Exp,
                                         scale=-1.0)
                    A.drain().then_inc(a2_sem, 1)

            @block.vector
            def _(vector):
                t = lambda n: FT[n][:]
                c4 = lambda n: C4[n][:]
                e8 = lambda n: E8[n][:]

                def sl(nm, i):
                    return E8[nm][:, i * F:(i + 1) * F]

                def sl4(nm, i):
                    return C4[nm][:, i * F:(i + 1) * F]

                def segreduce(dst, nm):
                    v = E8[nm][:].rearrange("p (s f) -> p f s", s=8)
                    V.tensor_reduce(dst, v, AX.X, OP.add)

                # constant zero slices of D8X/D8Y (box2's own AA edge dirs)
                V.memset(sl("D8X", 5), 0.0)
                V.memset(sl("D8X", 7), 0.0)
                V.memset(sl("D8Y", 4), 0.0)
                V.memset(sl("D8Y", 6), 0.0)

                for ch in range(NCHUNK):
                    pt, tg, wt = pt2[ch], tg2[ch], wt2[ch]
                    x1, y1, w1, h1, a1 = (pt[:, :, i] for i in range(5))
                    x2, y2, w2, h2, a2 = (tg[:, :, i] for i in range(5))

                    vector.wait_ge(dma_sem, 48 * (ch + 1))
                    # angles for ACT: delta, and wrapped a2+pi/2 in tmpA
                    V.tensor_tensor(t("delta"), a1, a2, OP.subtract)
                    V.tensor_scalar(t("tmpA"), a2, HALF_PI, None, OP.add)
                    V.tensor_scalar(t("tmpB"), t("tmpA"), float(np.pi), None,
                                    OP.is_gt)
                    V.scalar_tensor_tensor(t("tmpA"), t("tmpB"),
                                           float(-2 * np.pi), t("tmpA"),
                                           OP.mult, OP.add)
                    V.drain().then_inc(v_sem, 1)

                    # trig-independent work
                    V.tensor_tensor(t("px"), x2, x1, OP.subtract)
                    V.tensor_tensor(t("py"), y2, y1, OP.subtract)
                    V.tensor_scalar(t("W1"), w1, 0.5, None, OP.mult)
                    V.tensor_scalar(t("H1"), h1, 0.5, None, OP.mult)
                    V.tensor_scalar(t("W2"), w2, 0.5, None, OP.mult)
                    V.tensor_scalar(t("H2"), h2, 0.5, None, OP.mult)
                    V.tensor_tensor(t("ar1"), w1, h1, OP.mult)
                    V.tensor_tensor(t("ar2"), w2, h2, OP.mult)
                    V.tensor_scalar(t("ar2h"), t("ar2"), 0.5, None, OP.mult)
                    # box2 own corners -> K8X/K8Y slices 4..7
                    V.tensor_copy(sl("K8X", 4), t("W2"))
                    V.tensor_scalar(sl("K8X", 5), t("W2"), -1.0, None, OP.mult)
                    V.tensor_copy(sl("K8X", 6), sl("K8X", 5))
                    V.tensor_copy(sl("K8X", 7), t("W2"))
                    V.tensor_copy(sl("K8Y", 4), t("H2"))
                    V.tensor_copy(sl("K8Y", 5), t("H2"))
                    V.tensor_scalar(sl("K8Y", 6), t("H2"), -1.0, None, OP.mult)
                    V.tensor_copy(sl("K8Y", 7), sl("K8Y", 6))
                    # box2 own edge dirs -> D8X/D8Y slices 4..7 (x: -w2,0,w2,0)
                    V.tensor_scalar(sl("D8X", 4), w2, -1.0, None, OP.mult)
                    V.tensor_scalar(sl("D8X", 6), sl("D8X", 4), -1.0, None,
                                    OP.mult)
                    V.tensor_scalar(sl("D8Y", 5), h2, -1.0, None, OP.mult)
                    V.tensor_scalar(sl("D8Y", 7), sl("D8Y", 5), -1.0, None,
                                    OP.mult)

                    # ---- trig-dependent ------------------------------------
                    vector.wait_ge(a_sem, ch + 1)
                    cdA, s1A, c1A, s2A, c2A = (cd_t[:], s1_t[:], c1_t[:],
                                               s2_t[:], c2_t[:])
                    V.tensor_scalar(t("sg"), sdr_t[:], 0.0, None, OP.is_ge)
                    V.tensor_scalar(t("sg"), t("sg"), 2.0, -1.0, OP.mult,
                                    OP.add)
                    V.tensor_tensor(t("asd"), t("sg"), sdr_t[:], OP.mult)
                    V.tensor_scalar(t("asd"), t("asd"), TINY, None, OP.max)
                    V.tensor_tensor(t("sd"), t("sg"), t("asd"), OP.mult)
                    sdA = t("sd")

                    # box1 full products straight into D8 slices 0..3
                    wcf = sl("D8X", 2); V.tensor_tensor(wcf, w1, cdA, OP.mult)
                    V.tensor_scalar(sl("D8X", 0), wcf, -1.0, None, OP.mult)
                    wsf = sl("D8Y", 2); V.tensor_tensor(wsf, w1, sdA, OP.mult)
                    V.tensor_scalar(sl("D8Y", 0), wsf, -1.0, None, OP.mult)
                    hsf = sl("D8X", 1); V.tensor_tensor(hsf, h1, sdA, OP.mult)
                    V.tensor_scalar(sl("D8X", 3), hsf, -1.0, None, OP.mult)
                    hcf = sl("D8Y", 3); V.tensor_tensor(hcf, h1, cdA, OP.mult)
                    V.tensor_scalar(sl("D8Y", 1), hcf, -1.0, None, OP.mult)
                    # box2 full products (kept as F tiles; clip-B recips)
                    V.tensor_tensor(t("vcf"), w2, cdA, OP.mult)
                    V.tensor_tensor(t("vsf"), w2, sdA, OP.mult)
                    V.tensor_tensor(t("gcf"), h2, cdA, OP.mult)
                    V.tensor_tensor(t("gsf"), h2, sdA, OP.mult)
                    # |.| products for the ACT reciprocals of signed inputs
                    V.tensor_tensor(t("awsf"), w1, t("asd"), OP.mult)
                    V.tensor_tensor(t("ahsf"), h1, t("asd"), OP.mult)
                    V.tensor_tensor(t("avsf"), w2, t("asd"), OP.mult)
                    V.tensor_tensor(t("agsf"), h2, t("asd"), OP.mult)
                    V.drain().then_inc(v2_sem, 1)

                    # centers
                    V.tensor_tensor(t("tmpA"), t("px"), c2A, OP.mult)
                    V.tensor_tensor(t("tmpB"), t("py"), s2A, OP.mult)
                    V.scalar_tensor_tensor(t("mu"), t("tmpA"), -1.0, t("tmpB"),
                                           OP.mult, OP.subtract)
                    V.tensor_tensor(t("tmpA"), t("px"), s2A, OP.mult)
                    V.tensor_tensor(t("tmpB"), t("py"), c2A, OP.mult)
                    V.tensor_tensor(t("mv"), t("tmpA"), t("tmpB"), OP.subtract)
                    V.tensor_tensor(t("tmpA"), t("px"), c1A, OP.mult)
                    V.tensor_tensor(t("tmpB"), t("py"), s1A, OP.mult)
                    V.tensor_tensor(t("nu"), t("tmpA"), t("tmpB"), OP.add)
                    V.tensor_tensor(t("tmpA"), t("px"), s1A, OP.mult)
                    V.tensor_tensor(t("tmpB"), t("py"), c1A, OP.mult)
                    V.tensor_tensor(t("nv"), t("tmpB"), t("tmpA"), OP.subtract)

                    # box1 corners in box2 frame: half-combos in place, then
                    # pure-bf16 adds/subs (2x DVE mode; stt has none)
                    V.tensor_tensor(t("E1f"), wcf, hsf, OP.subtract)
                    V.tensor_tensor(t("E2f"), wcf, hsf, OP.add)
                    V.tensor_tensor(t("F1f"), wsf, hcf, OP.add)
                    V.tensor_tensor(t("F2f"), wsf, hcf, OP.subtract)
                    V.tensor_scalar(t("E1f"), t("E1f"), 0.5, None, OP.mult)
                    V.tensor_scalar(t("E2f"), t("E2f"), 0.5, None, OP.mult)
                    V.tensor_scalar(t("F1f"), t("F1f"), 0.5, None, OP.mult)
                    V.tensor_scalar(t("F2f"), t("F2f"), 0.5, None, OP.mult)
                    V.tensor_tensor(sl("K8X", 0), t("mu"), t("E1f"), OP.add)
                    V.tensor_tensor(sl("K8X", 1), t("mu"), t("E2f"), OP.subtract)
                    V.tensor_tensor(sl("K8X", 2), t("mu"), t("E1f"), OP.subtract)
                    V.tensor_tensor(sl("K8X", 3), t("mu"), t("E2f"), OP.add)
                    V.tensor_tensor(sl("K8Y", 0), t("mv"), t("F1f"), OP.add)
                    V.tensor_tensor(sl("K8Y", 1), t("mv"), t("F2f"), OP.subtract)
                    V.tensor_tensor(sl("K8Y", 2), t("mv"), t("F1f"), OP.subtract)
                    V.tensor_tensor(sl("K8Y", 3), t("mv"), t("F2f"), OP.add)
                    # box2 corners in box1 frame
                    V.tensor_tensor(t("P1f"), t("vcf"), t("gsf"), OP.add)
                    V.tensor_tensor(t("P2f"), t("vcf"), t("gsf"), OP.subtract)
                    V.tensor_tensor(t("Q1f"), t("gcf"), t("vsf"), OP.subtract)
                    V.tensor_tensor(t("Q2f"), t("gcf"), t("vsf"), OP.add)
                    V.tensor_scalar(t("P1f"), t("P1f"), 0.5, None, OP.mult)
                    V.tensor_scalar(t("P2f"), t("P2f"), 0.5, None, OP.mult)
                    V.tensor_scalar(t("Q1f"), t("Q1f"), 0.5, None, OP.mult)
                    V.tensor_scalar(t("Q2f"), t("Q2f"), 0.5, None, OP.mult)
                    V.tensor_tensor(sl4("GX4", 0), t("nu"), t("P1f"), OP.add)
                    V.tensor_tensor(sl4("GX4", 1), t("nu"), t("P2f"), OP.subtract)
                    V.tensor_tensor(sl4("GX4", 2), t("nu"), t("P1f"), OP.subtract)
                    V.tensor_tensor(sl4("GX4", 3), t("nu"), t("P2f"), OP.add)
                    V.tensor_tensor(sl4("GY4", 0), t("nv"), t("Q1f"), OP.add)
                    V.tensor_tensor(sl4("GY4", 1), t("nv"), t("Q2f"), OP.add)
                    V.tensor_tensor(sl4("GY4", 2), t("nv"), t("Q1f"), OP.subtract)
                    V.tensor_tensor(sl4("GY4", 3), t("nv"), t("Q2f"), OP.subtract)

                    def b3(ft):
                        # [P,F] -> broadcast [P,4,F]
                        return ft.rearrange("p (o f) -> p o f", o=1)\
                                 .to_broadcast((P, 4, F))

                    def v3(ap4):
                        return ap4.rearrange("p (s f) -> p s f", s=4)

                    vector.wait_ge(a2_sem, ch + 1)

                    def emit_clip(corner_x4, corner_y4, Wb, Hb,
                                  RXn, RYn, sgn_slots,
                                  lo_out, hi_out, span_out):
                        # shifted corners, batched with broadcast bounds
                        V.tensor_tensor(v3(c4("KXP")), v3(corner_x4), b3(Wb),
                                        OP.add)
                        V.tensor_tensor(v3(c4("KXM")), v3(corner_x4), b3(Wb),
                                        OP.subtract)
                        V.tensor_tensor(v3(c4("KYP")), v3(corner_y4), b3(Hb),
                                        OP.add)
                        V.tensor_tensor(v3(c4("KYM")), v3(corner_y4), b3(Hb),
                                        OP.subtract)
                        # R tiles: ACT prefilled positive slices; apply sign
                        # to the signed ones in place, then fill negated slots
                        for Rt, (pos0, neg0, pos1, neg1, signed) in (
                                (RXn, sgn_slots[0]), (RYn, sgn_slots[1])):
                            for s in signed:
                                V.tensor_tensor(sl4(Rt, s), t("sg"),
                                                sl4(Rt, s), OP.mult)
                            V.tensor_scalar(sl4(Rt, neg0), sl4(Rt, pos0),
                                            -1.0, None, OP.mult)
                            V.tensor_scalar(sl4(Rt, neg1), sl4(Rt, pos1),
                                            -1.0, None, OP.mult)
                        V.tensor_tensor(c4("KXP"), c4("KXP"), c4(RXn), OP.mult)
                        V.tensor_tensor(c4("KXM"), c4("KXM"), c4(RXn), OP.mult)
                        V.tensor_tensor(c4("KYP"), c4("KYP"), c4(RYn), OP.mult)
                        V.tensor_tensor(c4("KYM"), c4("KYM"), c4(RYn), OP.mult)
                        V.tensor_tensor(c4(RXn), c4("KXP"), c4("KXM"), OP.min)
                        V.tensor_tensor(c4("KXP"), c4("KXP"), c4("KXM"), OP.max)
                        V.tensor_tensor(c4(RYn), c4("KYP"), c4("KYM"), OP.min)
                        V.tensor_tensor(c4("KYP"), c4("KYP"), c4("KYM"), OP.max)
                        V.tensor_tensor(lo_out, c4(RXn), c4(RYn), OP.max)
                        V.tensor_scalar(lo_out, lo_out, 0.0, None, OP.max)
                        V.tensor_tensor(hi_out, c4("KXP"), c4("KYP"), OP.min)
                        V.tensor_scalar(hi_out, hi_out, 1.0, None, OP.min)
                        V.tensor_tensor(span_out, hi_out, lo_out, OP.subtract)
                        V.tensor_scalar(span_out, span_out, 0.0, None, OP.max)

                    # part A: box1 edges vs AA box2
                    # RXA: pos0=0(wcf,+) neg->2; pos1=3(|hsf|,signed) neg->1
                    # RYA: pos0=0(|wsf|,signed) neg->2; pos1=1(hcf,+) neg->3
                    emit_clip(E8["K8X"][:, 0:4 * F], E8["K8Y"][:, 0:4 * F],
                              t("W2"), t("H2"), "RXA", "RYA",
                              ((0, 2, 3, 1, (3,)), (0, 2, 1, 3, (0,))),
                              E8["TLO"][:, 0:4 * F], E8["THI"][:, 0:4 * F],
                              E8["SPAN"][:, 0:4 * F])
                    # part B: box2 edges vs AA box1
                    # RXB: pos0=0(vcf,+) neg->2; pos1=1(|gsf|,signed) neg->3
                    # RYB: pos0=2(|vsf|,signed) neg->0; pos1=1(gcf,+) neg->3
                    emit_clip(c4("GX4"), c4("GY4"),
                              t("W1"), t("H1"), "RXB", "RYB",
                              ((0, 2, 1, 3, (1,)), (2, 0, 1, 3, (2,))),
                              E8["TLO"][:, 4 * F:8 * F],
                              E8["THI"][:, 4 * F:8 * F],
                              E8["SPAN"][:, 4 * F:8 * F])

                    # ---- area terms ----------------------------------------
                    # A half: numeric cross(k,d); B half: cross = 2*W2*H2
                    EAa = E8["EA"][:, 0:4 * F]
                    EBa = E8["EB"][:, 0:4 * F]
                    V.tensor_tensor(EAa, E8["K8X"][:, 0:4 * F],
                                    E8["D8Y"][:, 0:4 * F], OP.mult)
                    V.tensor_tensor(EBa, E8["K8Y"][:, 0:4 * F],
                                    E8["D8X"][:, 0:4 * F], OP.mult)
                    V.tensor_tensor(EAa, EAa, EBa, OP.subtract)
                    V.tensor_tensor(EBa, E8["SPAN"][:, 0:4 * F], EAa, OP.mult)
                    V.tensor_tensor(
                        E8["EB"][:, 4 * F:8 * F].rearrange("p (s f) -> p s f", s=4),
                        E8["SPAN"][:, 4 * F:8 * F].rearrange("p (s f) -> p s f", s=4),
                        b3(t("ar2h")), OP.mult)
                    # EB = SV8 (span * cross) for all 8 segments

                    # ---- vertex centroid -----------------------------------
                    V.tensor_scalar(e8("EC"), e8("SPAN"), 0.0, None, OP.is_gt)
                    segreduce(t("m_"), "EC")
                    V.tensor_tensor(e8("ED"), e8("TLO"), e8("THI"), OP.add)
                    V.tensor_tensor(e8("THI"), e8("ED"), e8("D8X"), OP.mult)
                    V.tensor_scalar(e8("EA"), e8("K8X"), 2.0, None, OP.mult)
                    V.tensor_tensor(e8("THI"), e8("EA"), e8("THI"), OP.add)
                    V.tensor_tensor(e8("THI"), e8("THI"), e8("EC"), OP.mult)
                    segreduce(t("sx"), "THI")
                    V.tensor_tensor(e8("THI"), e8("ED"), e8("D8Y"), OP.mult)
                    V.tensor_scalar(e8("EA"), e8("K8Y"), 2.0, None, OP.mult)
                    V.tensor_tensor(e8("THI"), e8("EA"), e8("THI"), OP.add)
                    V.tensor_tensor(e8("THI"), e8("THI"), e8("EC"), OP.mult)
                    segreduce(t("sy"), "THI")
                    V.tensor_scalar(t("i2m"), t("m_"), 2.0, 1.0, OP.mult,
                                    OP.max)
                    V.reciprocal(t("i2m"), t("i2m"))
                    V.tensor_tensor(t("cx"), t("sx"), t("i2m"), OP.mult)
                    V.tensor_tensor(t("cy"), t("sy"), t("i2m"), OP.mult)

                    # ---- global-y of starts/dirs (KGY->EC, DGY->ED) --------
                    V.tensor_tensor(sl("ED", 2), w1, s1A, OP.mult)   # wsg
                    V.tensor_scalar(sl("ED", 0), sl("ED", 2), -1.0, None, OP.mult)
                    V.tensor_tensor(sl("ED", 3), h1, c1A, OP.mult)   # hcg
                    V.tensor_scalar(sl("ED", 1), sl("ED", 3), -1.0, None, OP.mult)
                    V.tensor_tensor(sl("ED", 6), w2, s2A, OP.mult)   # vsg
                    V.tensor_scalar(sl("ED", 4), sl("ED", 6), -1.0, None, OP.mult)
                    V.tensor_tensor(sl("ED", 7), h2, c2A, OP.mult)   # gcg
                    V.tensor_scalar(sl("ED", 5), sl("ED", 7), -1.0, None, OP.mult)
                    # S combos reuse E1f..F2f
                    V.tensor_tensor(t("E1f"), sl("ED", 2), sl("ED", 3), OP.add)
                    V.tensor_tensor(t("E2f"), sl("ED", 3), sl("ED", 2), OP.subtract)
                    V.tensor_tensor(t("F1f"), sl("ED", 6), sl("ED", 7), OP.add)
                    V.tensor_tensor(t("F2f"), sl("ED", 7), sl("ED", 6), OP.subtract)
                    V.scalar_tensor_tensor(sl("EC", 0), t("E1f"), 0.5, t("py"), OP.mult, OP.subtract)
                    V.scalar_tensor_tensor(sl("EC", 1), t("E2f"), 0.5, t("py"), OP.mult, OP.subtract)
                    V.scalar_tensor_tensor(sl("EC", 2), t("E1f"), -0.5, t("py"), OP.mult, OP.subtract)
                    V.scalar_tensor_tensor(sl("EC", 3), t("E2f"), -0.5, t("py"), OP.mult, OP.subtract)
                    V.tensor_scalar(sl("EC", 4), t("F1f"), 0.5, None, OP.mult)
                    V.tensor_scalar(sl("EC", 5), t("F2f"), 0.5, None, OP.mult)
                    V.tensor_scalar(sl("EC", 6), t("F1f"), -0.5, None, OP.mult)
                    V.tensor_scalar(sl("EC", 7), t("F2f"), -0.5, None, OP.mult)
                    # centroid global-y (bf16 copies feed the 8F broadcasts)
                    V.tensor_copy(t("cx16"), t("cx"))
                    V.tensor_copy(t("cy16"), t("cy"))
                    V.tensor_tensor(t("tA16"), s2A, t("cx16"), OP.mult)
                    V.tensor_tensor(t("tB16"), c2A, t("cy16"), OP.mult)
                    V.tensor_tensor(t("cgy16"), t("tA16"), t("tB16"), OP.add)

                    # ---- tau tests + correction ----------------------------
                    # TAU_A -> THI, KGYR -> EA (per slice), TAU_B -> TLO
                    def b8(ft):
                        return ft.rearrange("p (o f) -> p o f", o=1)\
                                 .to_broadcast((P, 8, F))

                    def v8(nm):
                        return E8[nm][:].rearrange("p (s f) -> p s f", s=8)

                    V.tensor_tensor(e8("THI"), e8("TLO"), e8("ED"), OP.mult)
                    V.tensor_tensor(v8("EA"), v8("EC"), b8(t("cgy16")),
                                    OP.subtract)
                    V.tensor_tensor(e8("THI"), e8("THI"), e8("EA"), OP.add)
                    V.tensor_tensor(e8("TLO"), e8("SPAN"), e8("ED"), OP.mult)
                    V.tensor_tensor(e8("TLO"), e8("TLO"), e8("THI"), OP.add)
                    V.tensor_scalar(e8("EC"), e8("THI"), 0.0, None, OP.is_ge)
                    V.tensor_scalar(e8("THI"), e8("TLO"), 0.0, None, OP.is_lt)
                    V.tensor_tensor(e8("EC"), e8("EC"), e8("THI"), OP.mult)
                    # chi = cross(c, d): EA = cx*D8Y, THI = cy*D8X (per slice)
                    V.tensor_tensor(v8("EA"), b8(t("cx16")), v8("D8Y"),
                                    OP.mult)
                    V.tensor_tensor(v8("THI"), b8(t("cy16")), v8("D8X"),
                                    OP.mult)
                    V.tensor_tensor(e8("EA"), e8("EA"), e8("THI"), OP.subtract)
                    V.tensor_tensor(e8("EA"), e8("SPAN"), e8("EA"), OP.mult)
                    V.tensor_tensor(e8("EA"), e8("EB"), e8("EA"), OP.subtract)
                    V.tensor_tensor(e8("EA"), e8("EC"), e8("EA"), OP.mult)

                    segreduce(t("S_"), "EB")
                    segreduce(t("CR_"), "EA")
                    V.tensor_tensor(t("S_"), t("S_"), t("CR_"), OP.subtract)

                    # ---- iou / loss ----------------------------------------
                    V.tensor_scalar(t("inter"), t("S_"), 0.5, 0.0, OP.mult,
                                    OP.max)
                    V.tensor_tensor(t("un"), t("ar1"), t("ar2"), OP.add)
                    V.tensor_tensor(t("un"), t("un"), t("inter"), OP.subtract)
                    V.tensor_scalar(t("un"), t("un"), ALPHA_EPS, None, OP.max)
                    V.reciprocal(t("un"), t("un"))
                    V.tensor_tensor(t("iou"), t("inter"), t("un"), OP.mult)
                    V.tensor_scalar(t("iou"), t("iou"), ALPHA_EPS, None, OP.max)
                    V.tensor_tensor(t("io2"), t("iou"), t("iou"), OP.mult)
                    V.tensor_tensor(t("io3"), t("io2"), t("iou"), OP.mult)
                    V.tensor_tensor(t("junk"), t("io3"), wt[:], OP.mult)
                    V.tensor_reduce(acc4[:, ch:ch + 1], t("junk"), AX.X, OP.add)
                    if ch == NCHUNK - 1:
                        V.drain().then_inc(done_sem, 1)

    return nc


def _get_graph():
    if "nc" not in _cache:
        _cache["nc"] = _build_graph()
    return _cache["nc"]


def _shard_inputs(pred, target, weight):
    """Pad to NCORES*NCORE boxes and split per core."""
    per = NSHARD
    pads = NCORE - per
    pad_box = np.zeros((pads, 5), np.float32)
    pad_box[:, 2] = 1.0
    pad_box[:, 3] = 1.0
    pad_box[:, 4] = 0.3
    in_maps = []
    for c in range(NCORES):
        lo, hi = c * per, (c + 1) * per
        p = np.concatenate([np.ascontiguousarray(pred[lo:hi]), pad_box], 0)
        t = np.concatenate([np.ascontiguousarray(target[lo:hi]), pad_box], 0)
        w = np.concatenate([np.ascontiguousarray(weight[lo:hi]),
                            np.zeros(pads, np.float32)], 0)
        in_maps.append({"pred": p, "target": t, "weight": w})
    return in_maps


def kernel(pred, target, weight):
    from concourse.bass_utils import run_bass_kernel_spmd

    pred = np.asarray(pred, np.float32)
    target = np.asarray(target, np.float32)
    weight = np.asarray(weight, np.float32)

    nc = _get_graph()
    in_maps = _shard_inputs(pred, target, weight)
    res = run_bass_kernel_spmd(nc, in_maps, list(range(NCORES)))
    _cache["last_result"] = res
    total = sum(float(r["out"].astype(np.float64).sum()) for r in res.results)
    wsum = float(weight.astype(np.float64).sum())
    loss = (wsum - total) / NGLOB
    return np.float32(loss)



# revision 17
# speedup vs baseline: 1.4914x; 1.0519x over previous
"""AlphaRotatedIoULoss on 8 TRN2 NeuronCores (raw Bass SPMD kernel, v2).

Sort-free replication of the reference's rotated-IoU loss:
  - intersection area via directed-segment shoelace: clip each box's 4 edges
    against the other box (branch-free Liang-Barsky in that box's local
    frame), then sum span*cross(k,d) over the 8 directed boundary segments
    (all expressed in box2's frame).
  - the reference's shoelace drops the closing edge (last->first angle-sorted
    vertex) because invalid candidate slots are zeroed; the missing term is
    the cross of the unique boundary segment crossing the global -x ray from
    the vertex centroid.  Replicated branch-free via a global-y sign test.
Data-parallel over boxes: each core takes 1/8th, emits per-partition
per-chunk partial sums of weight*iou^3; host:  loss = (sum(w) - total) / n.

v2: F=248 (2 chunks) with manually aliased SBUF scratch, scalings folded
into scalar_tensor_tensor ops, product negations written straight into the
segment-direction slices, analytic cross terms for box2's own (axis-aligned)
edges.
"""
import numpy as np

P = 128          # partitions
T = 496          # boxes per partition per core
F = 248          # chunk width (free dim)
NCHUNK = T // F  # 2
NCORE = P * T    # 63488 boxes per core
NSHARD = 62500   # real boxes per core
NGLOB = 500000
NCORES = 8
ALPHA_EPS = 1e-6
TINY = 1e-12

_cache = {}


def _build_graph():
    import concourse.bass as bass
    import concourse.mybir as mybir
    from contextlib import ExitStack

    dt = mybir.dt.float32
    dt16 = mybir.dt.bfloat16
    AF = mybir.ActivationFunctionType
    OP = mybir.AluOpType
    AX = mybir.AxisListType
    HALF_PI = float(np.pi / 2)

    nc = bass.Bass(detect_race_conditions=False)

    def reg_const(value):
        t = nc.alloc_sbuf_tensor(f"const-f32-{value}", [128, 1], dt)
        nc.gpsimd.memset(t.ap(), value)
        nc.const_aps.aps[(dt, float(value))] = t.ap()

    reg_const(HALF_PI)
    nc.all_engine_barrier()

    pred = nc.declare_dram_parameter("pred", [NCORE, 5], dt, isOutput=False)
    targ = nc.declare_dram_parameter("target", [NCORE, 5], dt, isOutput=False)
    wgt = nc.declare_dram_parameter("weight", [NCORE], dt, isOutput=False)
    out = nc.declare_dram_parameter("out", [P, NCHUNK], dt, isOutput=True)

    predv = pred.rearrange("(p t) f -> p t f", p=P)
    targv = targ.rearrange("(p t) f -> p t f", p=P)
    wv = wgt.rearrange("(p t) -> p t", p=P)

    V = nc.vector
    A = nc.scalar

    with ExitStack() as ctx:
        _n = [0]

        def alloc(shape, dtype=dt):
            _n[0] += 1
            return ctx.enter_context(
                nc.sbuf_tensor(f"tile{_n[0]}", shape, dtype))

        # double-buffered inputs (2 chunks -> one buffer each, no recycling)
        pt2 = [alloc([P, F, 5]) for _ in range(2)]
        tg2 = [alloc([P, F, 5]) for _ in range(2)]
        wt2 = [alloc([P, F]) for _ in range(2)]
        # ACT-owned trig outputs (bf16: feed DVE 2x-mode tensor ops)
        sdr_t, cd_t, s1_t, c1_t, s2_t, c2_t = (
            alloc([P, F], dt16) for _ in range(6))
        ltmp = [alloc([P, F]) for _ in range(4)]  # fp32: ln/exp log-domain
        # F-sized DVE scratch.  Geometry-magnitude tiles go bf16 (2x/4x DVE
        # modes); fp32 for trig args, reduce outputs, and the loss tail.
        FN32 = ("delta tmpA tmpB ar1 ar2 "
                "m_ sx sy i2m cx cy S_ CR_ inter un iou io2 io3 junk").split()
        FN16 = ("W1 H1 W2 H2 ar2h sg asd sd px py w116 h116 w216 h216 "
                "vcf vsf gcf gsf mu mv nu nv E1f E2f F1f F2f P1f P2f Q1f Q2f "
                "awsf ahsf avsf agsf cx16 cy16 cgy16 tA16 tB16").split()
        FT = {n: alloc([P, F]) for n in FN32}
        FT.update({n: alloc([P, F], dt16) for n in FN16})
        # 4F tiles
        C4 = {n: alloc([P, 4 * F], dt16) for n in
              "KXP KXM KYP KYM RXA RYA RXB RYB GX4 GY4".split()}
        # 8F tiles
        E8 = {n: alloc([P, 8 * F], dt16) for n in
              "K8X K8Y D8X D8Y TLO THI SPAN EA EB EC ED".split()}
        acc4 = alloc([P, NCHUNK])

        with (
            nc.semaphore("dma_sem") as dma_sem,
            nc.semaphore("v_sem") as v_sem,
            nc.semaphore("a_sem") as a_sem,
            nc.semaphore("v2_sem") as v2_sem,
            nc.semaphore("a2_sem") as a2_sem,
            nc.semaphore("done_sem") as done_sem,
            nc.Block() as block,
        ):
            @block.sync
            def _(sync):
                for ch in range(NCHUNK):
                    sync.dma_start(
                        out=pt2[ch][:], in_=predv[:, ch * F:(ch + 1) * F, :]
                    ).then_inc(dma_sem, 16)
                    sync.dma_start(
                        out=tg2[ch][:], in_=targv[:, ch * F:(ch + 1) * F, :]
                    ).then_inc(dma_sem, 16)
                    sync.dma_start(
                        out=wt2[ch][:], in_=wv[:, ch * F:(ch + 1) * F]
                    ).then_inc(dma_sem, 16)
                sync.wait_ge(done_sem, 1)
                sync.dma_start(out=out[:], in_=acc4[:]).then_inc(dma_sem, 16)

            @block.scalar
            def _(scalar):
                for ch in range(NCHUNK):
                    a1 = pt2[ch][:, :, 4]
                    a2 = tg2[ch][:, :, 4]
                    scalar.wait_ge(v_sem, ch + 1)
                    A.activation(sdr_t[:], FT["delta"][:], AF.Sin)
                    A.activation(cd_t[:], FT["delta"][:], AF.Sin, bias=HALF_PI)
                    A.activation(s1_t[:], a1, AF.Sin)
                    A.activation(c1_t[:], a1, AF.Sin, bias=HALF_PI)
                    A.activation(s2_t[:], a2, AF.Sin)
                    A.activation(c2_t[:], FT["tmpA"][:], AF.Sin)
                    A.drain().then_inc(a_sem, 1)
                    scalar.wait_ge(v2_sem, ch + 1)
                    # clip reciprocals: r = exp(-ln(x)), x > 0
                    wcf_s = E8["D8X"][:, 2 * F:3 * F]
                    hcf_s = E8["D8Y"][:, 3 * F:4 * F]
                    for ins, outs in (
                        ((wcf_s, FT["ahsf"][:], FT["awsf"][:], hcf_s),
                         (C4["RXA"][:, 0:F], C4["RXA"][:, 3 * F:4 * F],
                          C4["RYA"][:, 0:F], C4["RYA"][:, F:2 * F])),
                        ((FT["vcf"][:], FT["agsf"][:], FT["avsf"][:],
                          FT["gcf"][:]),
                         (C4["RXB"][:, 0:F], C4["RXB"][:, F:2 * F],
                          C4["RYB"][:, 2 * F:3 * F], C4["RYB"][:, F:2 * F])),
                    ):
                        for j in range(4):
                            A.activation(ltmp[j][:], ins[j], AF.Ln)
                        for j in range(4):
                            A.activation(outs[j], ltmp[j][:], AF.Exp,
                                         scale=-1.0)
                    A.drain().then_inc(a2_sem, 1)

            @block.vector
            def _(vector):
                t = lambda n: FT[n][:]
                c4 = lambda n: C4[n][:]
                e8 = lambda n: E8[n][:]

                def sl(nm, i):
                    return E8[nm][:, i * F:(i + 1) * F]

                def sl4(nm, i):
                    return C4[nm][:, i * F:(i + 1) * F]

                def segreduce(dst, nm):
                    v = E8[nm][:].rearrange("p (s f) -> p f s", s=8)
                    V.tensor_reduce(dst, v, AX.X, OP.add)

                # constant zero slices of D8X/D8Y (box2's own AA edge dirs)
                V.memset(sl("D8X", 5), 0.0)
                V.memset(sl("D8X", 7), 0.0)
                V.memset(sl("D8Y", 4), 0.0)
                V.memset(sl("D8Y", 6), 0.0)

                for ch in range(NCHUNK):
                    pt, tg, wt = pt2[ch], tg2[ch], wt2[ch]
                    x1, y1, w1, h1, a1 = (pt[:, :, i] for i in range(5))
                    x2, y2, w2, h2, a2 = (tg[:, :, i] for i in range(5))

                    vector.wait_ge(dma_sem, 48 * (ch + 1))
                    # angles for ACT: delta, and wrapped a2+pi/2 in tmpA
                    V.tensor_tensor(t("delta"), a1, a2, OP.subtract)
                    V.tensor_scalar(t("tmpA"), a2, HALF_PI, None, OP.add)
                    V.tensor_scalar(t("tmpB"), t("tmpA"), float(np.pi), None,
                                    OP.is_gt)
                    V.scalar_tensor_tensor(t("tmpA"), t("tmpB"),
                                           float(-2 * np.pi), t("tmpA"),
                                           OP.mult, OP.add)
                    V.drain().then_inc(v_sem, 1)

                    # trig-independent work (bf16 copies of w/h feed 2x mults)
                    V.tensor_tensor(t("px"), x2, x1, OP.subtract)
                    V.tensor_tensor(t("py"), y2, y1, OP.subtract)
                    V.tensor_copy(t("w116"), w1)
                    V.tensor_copy(t("h116"), h1)
                    V.tensor_copy(t("w216"), w2)
                    V.tensor_copy(t("h216"), h2)
                    V.tensor_scalar(t("W1"), w1, 0.5, None, OP.mult)
                    V.tensor_scalar(t("H1"), h1, 0.5, None, OP.mult)
                    V.tensor_scalar(t("W2"), w2, 0.5, None, OP.mult)
                    V.tensor_scalar(t("H2"), h2, 0.5, None, OP.mult)
                    V.tensor_tensor(t("ar1"), w1, h1, OP.mult)
                    V.tensor_tensor(t("ar2"), w2, h2, OP.mult)
                    V.tensor_scalar(t("ar2h"), t("ar2"), 0.5, None, OP.mult)
                    # box2 own corners -> K8X/K8Y slices 4..7
                    V.tensor_copy(sl("K8X", 4), t("W2"))
                    V.tensor_scalar(sl("K8X", 5), t("W2"), -1.0, None, OP.mult)
                    V.tensor_copy(sl("K8X", 6), sl("K8X", 5))
                    V.tensor_copy(sl("K8X", 7), t("W2"))
                    V.tensor_copy(sl("K8Y", 4), t("H2"))
                    V.tensor_copy(sl("K8Y", 5), t("H2"))
                    V.tensor_scalar(sl("K8Y", 6), t("H2"), -1.0, None, OP.mult)
                    V.tensor_copy(sl("K8Y", 7), sl("K8Y", 6))
                    # box2 own edge dirs -> D8X/D8Y slices 4..7 (x: -w2,0,w2,0)
                    V.tensor_scalar(sl("D8X", 4), t("w216"), -1.0, None,
                                    OP.mult)
                    V.tensor_scalar(sl("D8X", 6), sl("D8X", 4), -1.0, None,
                                    OP.mult)
                    V.tensor_scalar(sl("D8Y", 5), t("h216"), -1.0, None,
                                    OP.mult)
                    V.tensor_scalar(sl("D8Y", 7), sl("D8Y", 5), -1.0, None,
                                    OP.mult)

                    # ---- trig-dependent ------------------------------------
                    vector.wait_ge(a_sem, ch + 1)
                    cdA, s1A, c1A, s2A, c2A = (cd_t[:], s1_t[:], c1_t[:],
                                               s2_t[:], c2_t[:])
                    V.tensor_scalar(t("sg"), sdr_t[:], 0.0, None, OP.is_ge)
                    V.tensor_scalar(t("sg"), t("sg"), 2.0, -1.0, OP.mult,
                                    OP.add)
                    V.tensor_tensor(t("asd"), t("sg"), sdr_t[:], OP.mult)
                    V.tensor_scalar(t("asd"), t("asd"), TINY, None, OP.max)
                    V.tensor_tensor(t("sd"), t("sg"), t("asd"), OP.mult)
                    sdA = t("sd")

                    # box1 full products straight into D8 slices 0..3
                    wcf = sl("D8X", 2)
                    V.tensor_tensor(wcf, t("w116"), cdA, OP.mult)
                    V.tensor_scalar(sl("D8X", 0), wcf, -1.0, None, OP.mult)
                    wsf = sl("D8Y", 2)
                    V.tensor_tensor(wsf, t("w116"), sdA, OP.mult)
                    V.tensor_scalar(sl("D8Y", 0), wsf, -1.0, None, OP.mult)
                    hsf = sl("D8X", 1)
                    V.tensor_tensor(hsf, t("h116"), sdA, OP.mult)
                    V.tensor_scalar(sl("D8X", 3), hsf, -1.0, None, OP.mult)
                    hcf = sl("D8Y", 3)
                    V.tensor_tensor(hcf, t("h116"), cdA, OP.mult)
                    V.tensor_scalar(sl("D8Y", 1), hcf, -1.0, None, OP.mult)
                    # box2 full products (kept as F tiles; clip-B recips)
                    V.tensor_tensor(t("vcf"), t("w216"), cdA, OP.mult)
                    V.tensor_tensor(t("vsf"), t("w216"), sdA, OP.mult)
                    V.tensor_tensor(t("gcf"), t("h216"), cdA, OP.mult)
                    V.tensor_tensor(t("gsf"), t("h216"), sdA, OP.mult)
                    # |.| products for the ACT reciprocals of signed inputs
                    V.tensor_tensor(t("awsf"), t("w116"), t("asd"), OP.mult)
                    V.tensor_tensor(t("ahsf"), t("h116"), t("asd"), OP.mult)
                    V.tensor_tensor(t("avsf"), t("w216"), t("asd"), OP.mult)
                    V.tensor_tensor(t("agsf"), t("h216"), t("asd"), OP.mult)
                    V.drain().then_inc(v2_sem, 1)

                    # centers (all-bf16 2x ops via tA16/tB16 scratch)
                    V.tensor_tensor(t("tA16"), t("px"), c2A, OP.mult)
                    V.tensor_tensor(t("tB16"), t("py"), s2A, OP.mult)
                    V.tensor_tensor(t("mu"), t("tA16"), t("tB16"), OP.add)
                    V.tensor_scalar(t("mu"), t("mu"), -1.0, None, OP.mult)
                    V.tensor_tensor(t("tA16"), t("px"), s2A, OP.mult)
                    V.tensor_tensor(t("tB16"), t("py"), c2A, OP.mult)
                    V.tensor_tensor(t("mv"), t("tA16"), t("tB16"), OP.subtract)
                    V.tensor_tensor(t("tA16"), t("px"), c1A, OP.mult)
                    V.tensor_tensor(t("tB16"), t("py"), s1A, OP.mult)
                    V.tensor_tensor(t("nu"), t("tA16"), t("tB16"), OP.add)
                    V.tensor_tensor(t("tA16"), t("px"), s1A, OP.mult)
                    V.tensor_tensor(t("tB16"), t("py"), c1A, OP.mult)
                    V.tensor_tensor(t("nv"), t("tB16"), t("tA16"), OP.subtract)

                    # box1 corners in box2 frame: half-combos in place, then
                    # pure-bf16 adds/subs (2x DVE mode; stt has none)
                    V.tensor_tensor(t("E1f"), wcf, hsf, OP.subtract)
                    V.tensor_tensor(t("E2f"), wcf, hsf, OP.add)
                    V.tensor_tensor(t("F1f"), wsf, hcf, OP.add)
                    V.tensor_tensor(t("F2f"), wsf, hcf, OP.subtract)
                    V.tensor_scalar(t("E1f"), t("E1f"), 0.5, None, OP.mult)
                    V.tensor_scalar(t("E2f"), t("E2f"), 0.5, None, OP.mult)
                    V.tensor_scalar(t("F1f"), t("F1f"), 0.5, None, OP.mult)
                    V.tensor_scalar(t("F2f"), t("F2f"), 0.5, None, OP.mult)
                    V.tensor_tensor(sl("K8X", 0), t("mu"), t("E1f"), OP.add)
                    V.tensor_tensor(sl("K8X", 1), t("mu"), t("E2f"), OP.subtract)
                    V.tensor_tensor(sl("K8X", 2), t("mu"), t("E1f"), OP.subtract)
                    V.tensor_tensor(sl("K8X", 3), t("mu"), t("E2f"), OP.add)
                    V.tensor_tensor(sl("K8Y", 0), t("mv"), t("F1f"), OP.add)
                    V.tensor_tensor(sl("K8Y", 1), t("mv"), t("F2f"), OP.subtract)
                    V.tensor_tensor(sl("K8Y", 2), t("mv"), t("F1f"), OP.subtract)
                    V.tensor_tensor(sl("K8Y", 3), t("mv"), t("F2f"), OP.add)
                    # box2 corners in box1 frame
                    V.tensor_tensor(t("P1f"), t("vcf"), t("gsf"), OP.add)
                    V.tensor_tensor(t("P2f"), t("vcf"), t("gsf"), OP.subtract)
                    V.tensor_tensor(t("Q1f"), t("gcf"), t("vsf"), OP.subtract)
                    V.tensor_tensor(t("Q2f"), t("gcf"), t("vsf"), OP.add)
                    V.tensor_scalar(t("P1f"), t("P1f"), 0.5, None, OP.mult)
                    V.tensor_scalar(t("P2f"), t("P2f"), 0.5, None, OP.mult)
                    V.tensor_scalar(t("Q1f"), t("Q1f"), 0.5, None, OP.mult)
                    V.tensor_scalar(t("Q2f"), t("Q2f"), 0.5, None, OP.mult)
                    V.tensor_tensor(sl4("GX4", 0), t("nu"), t("P1f"), OP.add)
                    V.tensor_tensor(sl4("GX4", 1), t("nu"), t("P2f"), OP.subtract)
                    V.tensor_tensor(sl4("GX4", 2), t("nu"), t("P1f"), OP.subtract)
                    V.tensor_tensor(sl4("GX4", 3), t("nu"), t("P2f"), OP.add)
                    V.tensor_tensor(sl4("GY4", 0), t("nv"), t("Q1f"), OP.add)
                    V.tensor_tensor(sl4("GY4", 1), t("nv"), t("Q2f"), OP.add)
                    V.tensor_tensor(sl4("GY4", 2), t("nv"), t("Q1f"), OP.subtract)
                    V.tensor_tensor(sl4("GY4", 3), t("nv"), t("Q2f"), OP.subtract)

                    def b3(ft):
                        # [P,F] -> broadcast [P,4,F]
                        return ft.rearrange("p (o f) -> p o f", o=1)\
                                 .to_broadcast((P, 4, F))

                    def v3(ap4):
                        return ap4.rearrange("p (s f) -> p s f", s=4)

                    vector.wait_ge(a2_sem, ch + 1)

                    def emit_clip(corner_x4, corner_y4, Wb, Hb,
                                  RXn, RYn, sgn_slots,
                                  lo_out, hi_out, span_out):
                        # shifted corners, batched with broadcast bounds
                        V.tensor_tensor(v3(c4("KXP")), v3(corner_x4), b3(Wb),
                                        OP.add)
                        V.tensor_tensor(v3(c4("KXM")), v3(corner_x4), b3(Wb),
                                        OP.subtract)
                        V.tensor_tensor(v3(c4("KYP")), v3(corner_y4), b3(Hb),
                                        OP.add)
                        V.tensor_tensor(v3(c4("KYM")), v3(corner_y4), b3(Hb),
                                        OP.subtract)
                        # R tiles: ACT prefilled positive slices; apply sign
                        # to the signed ones in place, then fill negated slots
                        for Rt, (pos0, neg0, pos1, neg1, signed) in (
                                (RXn, sgn_slots[0]), (RYn, sgn_slots[1])):
                            for s in signed:
                                V.tensor_tensor(sl4(Rt, s), t("sg"),
                                                sl4(Rt, s), OP.mult)
                            V.tensor_scalar(sl4(Rt, neg0), sl4(Rt, pos0),
                                            -1.0, None, OP.mult)
                            V.tensor_scalar(sl4(Rt, neg1), sl4(Rt, pos1),
                                            -1.0, None, OP.mult)
                        V.tensor_tensor(c4("KXP"), c4("KXP"), c4(RXn), OP.mult)
                        V.tensor_tensor(c4("KXM"), c4("KXM"), c4(RXn), OP.mult)
                        V.tensor_tensor(c4("KYP"), c4("KYP"), c4(RYn), OP.mult)
                        V.tensor_tensor(c4("KYM"), c4("KYM"), c4(RYn), OP.mult)
                        V.tensor_tensor(c4(RXn), c4("KXP"), c4("KXM"), OP.min)
                        V.tensor_tensor(c4("KXP"), c4("KXP"), c4("KXM"), OP.max)
                        V.tensor_tensor(c4(RYn), c4("KYP"), c4("KYM"), OP.min)
                        V.tensor_tensor(c4("KYP"), c4("KYP"), c4("KYM"), OP.max)
                        V.tensor_tensor(lo_out, c4(RXn), c4(RYn), OP.max)
                        V.tensor_scalar(lo_out, lo_out, 0.0, None, OP.max)
                        V.tensor_tensor(hi_out, c4("KXP"), c4("KYP"), OP.min)
                        V.tensor_scalar(hi_out, hi_out, 1.0, None, OP.min)
                        V.tensor_tensor(span_out, hi_out, lo_out, OP.subtract)
                        V.tensor_scalar(span_out, span_out, 0.0, None, OP.max)

                    # part A: box1 edges vs AA box2
                    # RXA: pos0=0(wcf,+) neg->2; pos1=3(|hsf|,signed) neg->1
                    # RYA: pos0=0(|wsf|,signed) neg->2; pos1=1(hcf,+) neg->3
                    emit_clip(E8["K8X"][:, 0:4 * F], E8["K8Y"][:, 0:4 * F],
                              t("W2"), t("H2"), "RXA", "RYA",
                              ((0, 2, 3, 1, (3,)), (0, 2, 1, 3, (0,))),
                              E8["TLO"][:, 0:4 * F], E8["THI"][:, 0:4 * F],
                              E8["SPAN"][:, 0:4 * F])
                    # part B: box2 edges vs AA box1
                    # RXB: pos0=0(vcf,+) neg->2; pos1=1(|gsf|,signed) neg->3
                    # RYB: pos0=2(|vsf|,signed) neg->0; pos1=1(gcf,+) neg->3
                    emit_clip(c4("GX4"), c4("GY4"),
                              t("W1"), t("H1"), "RXB", "RYB",
                              ((0, 2, 1, 3, (1,)), (2, 0, 1, 3, (2,))),
                              E8["TLO"][:, 4 * F:8 * F],
                              E8["THI"][:, 4 * F:8 * F],
                              E8["SPAN"][:, 4 * F:8 * F])

                    # ---- area terms ----------------------------------------
                    # A half: numeric cross(k,d); B half: cross = 2*W2*H2
                    EAa = E8["EA"][:, 0:4 * F]
                    EBa = E8["EB"][:, 0:4 * F]
                    V.tensor_tensor(EAa, E8["K8X"][:, 0:4 * F],
                                    E8["D8Y"][:, 0:4 * F], OP.mult)
                    V.tensor_tensor(EBa, E8["K8Y"][:, 0:4 * F],
                                    E8["D8X"][:, 0:4 * F], OP.mult)
                    V.tensor_tensor(EAa, EAa, EBa, OP.subtract)
                    V.tensor_tensor(EBa, E8["SPAN"][:, 0:4 * F], EAa, OP.mult)
                    V.tensor_tensor(
                        E8["EB"][:, 4 * F:8 * F].rearrange("p (s f) -> p s f", s=4),
                        E8["SPAN"][:, 4 * F:8 * F].rearrange("p (s f) -> p s f", s=4),
                        b3(t("ar2h")), OP.mult)
                    # EB = SV8 (span * cross) for all 8 segments

                    # ---- vertex centroid -----------------------------------
                    V.tensor_scalar(e8("EC"), e8("SPAN"), 0.0, None, OP.is_gt)
                    segreduce(t("m_"), "EC")
                    V.tensor_tensor(e8("ED"), e8("TLO"), e8("THI"), OP.add)
                    V.tensor_tensor(e8("THI"), e8("ED"), e8("D8X"), OP.mult)
                    V.tensor_scalar(e8("EA"), e8("K8X"), 2.0, None, OP.mult)
                    V.tensor_tensor(e8("THI"), e8("EA"), e8("THI"), OP.add)
                    V.tensor_tensor(e8("THI"), e8("THI"), e8("EC"), OP.mult)
                    segreduce(t("sx"), "THI")
                    V.tensor_tensor(e8("THI"), e8("ED"), e8("D8Y"), OP.mult)
                    V.tensor_scalar(e8("EA"), e8("K8Y"), 2.0, None, OP.mult)
                    V.tensor_tensor(e8("THI"), e8("EA"), e8("THI"), OP.add)
                    V.tensor_tensor(e8("THI"), e8("THI"), e8("EC"), OP.mult)
                    segreduce(t("sy"), "THI")
                    V.tensor_scalar(t("i2m"), t("m_"), 2.0, 1.0, OP.mult,
                                    OP.max)
                    V.reciprocal(t("i2m"), t("i2m"))
                    V.tensor_tensor(t("cx"), t("sx"), t("i2m"), OP.mult)
                    V.tensor_tensor(t("cy"), t("sy"), t("i2m"), OP.mult)

                    # ---- global-y of starts/dirs (KGY->EC, DGY->ED) --------
                    V.tensor_tensor(sl("ED", 2), t("w116"), s1A, OP.mult)
                    V.tensor_scalar(sl("ED", 0), sl("ED", 2), -1.0, None, OP.mult)
                    V.tensor_tensor(sl("ED", 3), t("h116"), c1A, OP.mult)
                    V.tensor_scalar(sl("ED", 1), sl("ED", 3), -1.0, None, OP.mult)
                    V.tensor_tensor(sl("ED", 6), t("w216"), s2A, OP.mult)
                    V.tensor_scalar(sl("ED", 4), sl("ED", 6), -1.0, None, OP.mult)
                    V.tensor_tensor(sl("ED", 7), t("h216"), c2A, OP.mult)
                    V.tensor_scalar(sl("ED", 5), sl("ED", 7), -1.0, None, OP.mult)
                    # S combos reuse E1f..F2f
                    V.tensor_tensor(t("E1f"), sl("ED", 2), sl("ED", 3), OP.add)
                    V.tensor_tensor(t("E2f"), sl("ED", 3), sl("ED", 2), OP.subtract)
                    V.tensor_tensor(t("F1f"), sl("ED", 6), sl("ED", 7), OP.add)
                    V.tensor_tensor(t("F2f"), sl("ED", 7), sl("ED", 6), OP.subtract)
                    V.scalar_tensor_tensor(sl("EC", 0), t("E1f"), 0.5, t("py"), OP.mult, OP.subtract)
                    V.scalar_tensor_tensor(sl("EC", 1), t("E2f"), 0.5, t("py"), OP.mult, OP.subtract)
                    V.scalar_tensor_tensor(sl("EC", 2), t("E1f"), -0.5, t("py"), OP.mult, OP.subtract)
                    V.scalar_tensor_tensor(sl("EC", 3), t("E2f"), -0.5, t("py"), OP.mult, OP.subtract)
                    V.tensor_scalar(sl("EC", 4), t("F1f"), 0.5, None, OP.mult)
                    V.tensor_scalar(sl("EC", 5), t("F2f"), 0.5, None, OP.mult)
                    V.tensor_scalar(sl("EC", 6), t("F1f"), -0.5, None, OP.mult)
                    V.tensor_scalar(sl("EC", 7), t("F2f"), -0.5, None, OP.mult)
                    # centroid global-y (bf16 copies feed the 8F broadcasts)
                    V.tensor_copy(t("cx16"), t("cx"))
                    V.tensor_copy(t("cy16"), t("cy"))
                    V.tensor_tensor(t("tA16"), s2A, t("cx16"), OP.mult)
                    V.tensor_tensor(t("tB16"), c2A, t("cy16"), OP.mult)
                    V.tensor_tensor(t("cgy16"), t("tA16"), t("tB16"), OP.add)

                    # ---- tau tests + correction ----------------------------
                    # TAU_A -> THI, KGYR -> EA (per slice), TAU_B -> TLO
                    def b8(ft):
                        return ft.rearrange("p (o f) -> p o f", o=1)\
                                 .to_broadcast((P, 8, F))

                    def v8(nm):
                        return E8[nm][:].rearrange("p (s f) -> p s f", s=8)

                    V.tensor_tensor(e8("THI"), e8("TLO"), e8("ED"), OP.mult)
                    V.tensor_tensor(v8("EA"), v8("EC"), b8(t("cgy16")),
                                    OP.subtract)
                    V.tensor_tensor(e8("THI"), e8("THI"), e8("EA"), OP.add)
                    V.tensor_tensor(e8("TLO"), e8("SPAN"), e8("ED"), OP.mult)
                    V.tensor_tensor(e8("TLO"), e8("TLO"), e8("THI"), OP.add)
                    V.tensor_scalar(e8("EC"), e8("THI"), 0.0, None, OP.is_ge)
                    V.tensor_scalar(e8("THI"), e8("TLO"), 0.0, None, OP.is_lt)
                    V.tensor_tensor(e8("EC"), e8("EC"), e8("THI"), OP.mult)
                    # chi = cross(c, d): EA = cx*D8Y, THI = cy*D8X (per slice)
                    V.tensor_tensor(v8("EA"), b8(t("cx16")), v8("D8Y"),
                                    OP.mult)
                    V.tensor_tensor(v8("THI"), b8(t("cy16")), v8("D8X"),
                                    OP.mult)
                    V.tensor_tensor(e8("EA"), e8("EA"), e8("THI"), OP.subtract)
                    V.tensor_tensor(e8("EA"), e8("SPAN"), e8("EA"), OP.mult)
                    V.tensor_tensor(e8("EA"), e8("EB"), e8("EA"), OP.subtract)
                    V.tensor_tensor(e8("EA"), e8("EC"), e8("EA"), OP.mult)

                    segreduce(t("S_"), "EB")
                    segreduce(t("CR_"), "EA")
                    V.tensor_tensor(t("S_"), t("S_"), t("CR_"), OP.subtract)

                    # ---- iou / loss ----------------------------------------
                    V.tensor_scalar(t("inter"), t("S_"), 0.5, 0.0, OP.mult,
                                    OP.max)
                    V.tensor_tensor(t("un"), t("ar1"), t("ar2"), OP.add)
                    V.tensor_tensor(t("un"), t("un"), t("inter"), OP.subtract)
                    V.tensor_scalar(t("un"), t("un"), ALPHA_EPS, None, OP.max)
                    V.reciprocal(t("un"), t("un"))
                    V.tensor_tensor(t("iou"), t("inter"), t("un"), OP.mult)
                    V.tensor_scalar(t("iou"), t("iou"), ALPHA_EPS, None, OP.max)
                    V.tensor_tensor(t("io2"), t("iou"), t("iou"), OP.mult)
                    V.tensor_tensor(t("io3"), t("io2"), t("iou"), OP.mult)
                    V.tensor_tensor(t("junk"), t("io3"), wt[:], OP.mult)
                    V.tensor_reduce(acc4[:, ch:ch + 1], t("junk"), AX.X, OP.add)
                    if ch == NCHUNK - 1:
                        V.drain().then_inc(done_sem, 1)

    return nc


def _get_graph():
    if "nc" not in _cache:
        _cache["nc"] = _build_graph()
    return _cache["nc"]


def _shard_inputs(pred, target, weight):
    """Pad to NCORES*NCORE boxes and split per core."""
    per = NSHARD
    pads = NCORE - per
    pad_box = np.zeros((pads, 5), np.float32)
    pad_box[:, 2] = 1.0
    pad_box[:, 3] = 1.0
    pad_box[:, 4] = 0.3
    in_maps = []
    for c in range(NCORES):
        lo, hi = c * per, (c + 1) * per
        p = np.concatenate([np.ascontiguousarray(pred[lo:hi]), pad_box], 0)
        t = np.concatenate([np.ascontiguousarray(target[lo:hi]), pad_box], 0)
        w = np.concatenate([np.ascontiguousarray(weight[lo:hi]),
                            np.zeros(pads, np.float32)], 0)
        in_maps.append({"pred": p, "target": t, "weight": w})
    return in_maps


def kernel(pred, target, weight):
    from concourse.bass_utils import run_bass_kernel_spmd

    pred = np.asarray(pred, np.float32)
    target = np.asarray(target, np.float32)
    weight = np.asarray(weight, np.float32)

    nc = _get_graph()
    in_maps = _shard_inputs(pred, target, weight)
    res = run_bass_kernel_spmd(nc, in_maps, list(range(NCORES)))
    _cache["last_result"] = res
    total = sum(float(r["out"].astype(np.float64).sum()) for r in res.results)
    wsum = float(weight.astype(np.float64).sum())
    loss = (wsum - total) / NGLOB
    return np.float32(loss)

